# revision 10
# baseline (speedup 1.0000x reference)
"""Equivariant interaction block (gnn message passing) on 8 trn2 NeuronCores.

v2 strategy (per-core, edges dst-sorted and sharded by dst node range):
  pass 1 per 512-edge supertile:
    - radial MLP on PE (bias via ones-row), h2s [65, 512] bf16
    - per 128-edge sub-tile: W' = h2s_c @ w3p in 5 PSUM chunks (rotating
      banks), Scalar copies -> W'sb [128, (sub, 2304)] bf16
    - indirect-gather x rows (144-col table: xs | gap | xv(d,i) | xv(i,d))
    - ALL elementwise work batched at supertile granularity on DVE:
      one scale-mult (host-shipped shpat), one small reduce, 3 product
      ops (broadcast APs), binary add-trees, 2 assembly ops
    - host-shipped selection matrices S fuse the within-tile segment-sum
      on PE; partials written per supertile
  pass 2 per 128-node block: 2 indirect gathers of partials, weighted
    combine (host-folded inverse degree), PE transpose + fused irrep
    linear (f32), write yT.
"""

import os
import sys

import numpy as np

for _p in ("/opt/trn_rl_repo", os.path.expanduser("~/.axon_site/_ro/trn_rl_repo")):
    if os.path.isdir(_p) and _p not in sys.path:
        sys.path.insert(0, _p)

import concourse.bacc as bacc
import concourse.bass as bass
import concourse.mybir as mybir
import concourse.tile as tile
from concourse.bass_utils import run_bass_kernel_spmd

F32 = mybir.dt.float32
BF16 = mybir.dt.bfloat16
I32 = mybir.dt.int32
AF = mybir.ActivationFunctionType
OP = mybir.AluOpType

MUL0, MUL1 = 32, 16
RBF, HID = 16, 64
O1 = MUL0 * MUL0
O2 = O1 + MUL0 * MUL1
O3 = O2 + MUL1 * MUL1
WNUMEL = O3 + MUL1 * MUL0  # 2304
C_PATH = float(1.0 / np.sqrt(np.float32(MUL0 + MUL1)))
C_110 = float(1.0 / np.sqrt(3.0))
NCORES = 8
FDIM = MUL0 + 3 * MUL1  # 80
XCOLS = 176  # xs*shs 32 | afb 16 | xv(d,i) 48 | xv(i,d) 48 | xs 32


def _build_w3_perm():
    """Permutation + scale taking reference w3 columns into our layout.

    W' columns (2304):
      A [0,1536):    q = o*48 + j      (o in 32, j in 48)
          j < 32 : path1  W1[i=j, o]   -> src i*32+o          scale C_PATH
          j >= 32: path4  W4[i=j-32,o] -> src O3 + i*32+o     scale C_PATH*C110
      B [1536,2048): q = 1536 + o*32 + i (o in 16, i in 32)
          path2 W2[i, o] -> src O1 + i*16 + o                 scale C_PATH
      C [2048,2304): q = 2048 + o*16 + i (o in 16, i in 16)
          path3 W3[i, o] -> src O2 + i*16 + o                 scale C_PATH
    """
    src = np.zeros(WNUMEL, np.int64)
    scl = np.zeros(WNUMEL, np.float32)
    for o in range(MUL0):
        for j in range(48):
            q = o * 48 + j
            if j < 32:
                src[q] = j * MUL0 + o
                scl[q] = C_PATH
            else:
                src[q] = O3 + (j - 32) * MUL0 + o
                scl[q] = C_PATH * C_110
    for o in range(MUL1):
        for i in range(MUL0):
            q = 1536 + o * 32 + i
            src[q] = O1 + i * MUL1 + o
            scl[q] = C_PATH
    for o in range(MUL1):
        for i in range(MUL1):
            q = 2048 + o * 16 + i
            src[q] = O2 + i * MUL1 + o
            scl[q] = C_PATH
    return src, scl


def _irrep_matrix_L(ws, wv):
    """[80,80] M in internal layout L = [s(32) | v(d,o): 32+d*16+o]."""
    M = np.zeros((FDIM, FDIM), np.float32)
    M[:MUL0, :MUL0] = ws
    for d in range(3):
        b = MUL0 + d * MUL1
        M[b:b + MUL1, b:b + MUL1] = wv
    return M


def build_program(n_nodes, npc_pad, epad, num_cores):
    nsuper = epad // 512
    nb = npc_pad // 128
    assert epad % 512 == 0 and npc_pad % 128 == 0

    nc = bacc.Bacc(
        "TRN2",
        target_bir_lowering=False,
        debug=False,
        enable_asserts=False,
        num_devices=num_cores,
    )

    x_tbl = nc.dram_tensor("x_tbl", [n_nodes, XCOLS], F32, kind="ExternalInput")
    xshT = nc.dram_tensor("xshT", [FDIM, npc_pad], F32, kind="ExternalInput")
    src_g = nc.dram_tensor("src_g", [nsuper * 128, 4], I32, kind="ExternalInput")
    shpat = nc.dram_tensor("shpat", [nsuper * 128, 4 * XCOLS], BF16,
                           kind="ExternalInput")
    shvrep = nc.dram_tensor("shvrep", [nsuper * 128, 4 * 48], BF16,
                            kind="ExternalInput")
    smat = nc.dram_tensor("smat", [nsuper * 128, 4 * 128], BF16,
                          kind="ExternalInput")
    rbf17 = nc.dram_tensor("rbf17", [RBF + 1, epad], BF16, kind="ExternalInput")
    w1b = nc.dram_tensor("w1b", [RBF + 1, HID], BF16, kind="ExternalInput")
    w2b = nc.dram_tensor("w2b", [HID + 1, HID], BF16, kind="ExternalInput")
    w3p = nc.dram_tensor("w3p", [HID + 1, WNUMEL], BF16, kind="ExternalInput")
    g12 = nc.dram_tensor("g12", [npc_pad, 2], I32, kind="ExternalInput")
    w12 = nc.dram_tensor("w12", [npc_pad, 2], F32, kind="ExternalInput")
    msf = nc.dram_tensor("msf", [FDIM, FDIM], F32, kind="ExternalInput")
    mout = nc.dram_tensor("mout", [FDIM, FDIM], F32, kind="ExternalInput")
    ident = nc.dram_tensor("ident", [128, 128], F32, kind="ExternalInput")

    yT = nc.dram_tensor("yT", [FDIM, npc_pad], F32, kind="ExternalOutput")
    partials = nc.dram_tensor("partials", [epad, FDIM], F32)

    with tile.TileContext(nc) as tc:
        with (
            nc.allow_low_precision(reason="bf16 per-edge messages, f32 agg"),
            tc.tile_pool(name="const", bufs=1) as cp,
        ):
            w1b_sb = cp.tile([RBF + 1, HID], BF16)
            nc.sync.dma_start(out=w1b_sb[:], in_=w1b[:])
            w2b_sb = cp.tile([HID + 1, HID], BF16)
            nc.sync.dma_start(out=w2b_sb[:], in_=w2b[:])
            w3p_sb = cp.tile([HID + 1, WNUMEL], BF16)
            nc.sync.dma_start(out=w3p_sb[:], in_=w3p[:])
            ident_sb = cp.tile([128, 128], F32)
            nc.sync.dma_start(out=ident_sb[:], in_=ident[:])
            msf_sb = cp.tile([FDIM, FDIM], F32)
            nc.sync.dma_start(out=msf_sb[:], in_=msf[:])
            mout_sb = cp.tile([FDIM, FDIM], F32)
            nc.sync.dma_start(out=mout_sb[:], in_=mout[:])

            # ---------------- pass 1: edges ----------------
            with (
                tc.tile_pool(name="sb", bufs=2) as sp,
                tc.tile_pool(name="wps", bufs=4, space="PSUM") as wpp,
                tc.tile_pool(name="mlp", bufs=2, space="PSUM") as mpp,
                tc.tile_pool(name="cmb", bufs=2, space="PSUM") as cpp,
            ):
                for s in range(nsuper):
                    r0 = s * 512
                    rbf_t = sp.tile([RBF + 1, 512], BF16, tag="rbf")
                    nc.sync.dma_start(out=rbf_t[:], in_=rbf17[:, r0:r0 + 512])
                    src_t = sp.tile([128, 4], I32, tag="src")
                    nc.sync.dma_start(out=src_t[:],
                                      in_=src_g[s * 128:(s + 1) * 128, :])
                    shp_t = sp.tile([128, 4 * XCOLS], BF16, tag="shp")
                    nc.sync.dma_start(out=shp_t[:],
                                      in_=shpat[s * 128:(s + 1) * 128, :])
                    shv_t = sp.tile([128, 4 * 48], BF16, tag="shv")
                    nc.sync.dma_start(out=shv_t[:],
                                      in_=shvrep[s * 128:(s + 1) * 128, :])
                    s_t = sp.tile([128, 4 * 128], BF16, tag="smat")
                    nc.sync.dma_start(out=s_t[:],
                                      in_=smat[s * 128:(s + 1) * 128, :])

                    # radial MLP
                    h1_ps = mpp.tile([HID, 512], F32, tag="mlp")
                    nc.tensor.matmul(out=h1_ps[:], lhsT=w1b_sb[:], rhs=rbf_t[:],
                                     start=True, stop=True)
                    h1s = sp.tile([HID + 1, 512], BF16, tag="h1s")
                    nc.scalar.activation(h1s[:HID, :], h1_ps[:], AF.Silu)
                    nc.gpsimd.memset(h1s[HID:HID + 1, :], 1.0)
                    h2_ps = mpp.tile([HID, 512], F32, tag="mlp")
                    nc.tensor.matmul(out=h2_ps[:], lhsT=w2b_sb[:], rhs=h1s[:],
                                     start=True, stop=True)
                    h2s = sp.tile([HID + 1, 512], BF16, tag="h2s")
                    nc.scalar.activation(h2s[:HID, :], h2_ps[:], AF.Silu)
                    nc.gpsimd.memset(h2s[HID:HID + 1, :], 1.0)

                    # gather + W' per sub-tile
                    xg = sp.tile([128, 4 * XCOLS], F32, tag="xg")
                    wsb = sp.tile([128, 4 * WNUMEL], BF16, tag="wsb")
                    for c in range(4):
                        nc.gpsimd.indirect_dma_start(
                            out=xg[:, c * XCOLS:(c + 1) * XCOLS],
                            out_offset=None, in_=x_tbl[:],
                            in_offset=bass.IndirectOffsetOnAxis(
                                ap=src_t[:, c:c + 1], axis=0))
                        lhs = h2s[:, c * 128:(c + 1) * 128]
                        for k in range(5):
                            q0 = k * 512
                            q1 = min(q0 + 512, WNUMEL)
                            w_ps = wpp.tile([128, 512], F32, tag="w")
                            nc.tensor.matmul(out=w_ps[:, :q1 - q0], lhsT=lhs,
                                             rhs=w3p_sb[:, q0:q1],
                                             start=True, stop=True)
                            nc.scalar.copy(
                                wsb[:, c * WNUMEL + q0:c * WNUMEL + q1],
                                w_ps[:, :q1 - q0])

                    # ---- supertile-batched DVE ----
                    # xft = xg * shpat  (f32 x bf16 -> bf16)
                    xft = sp.tile([128, 4 * XCOLS], BF16, tag="xft")
                    nc.vector.tensor_tensor(out=xft[:], in0=xg[:], in1=shp_t[:],
                                            op=OP.mult)
                    xft4 = xft[:].rearrange("p (s f) -> p s f", f=XCOLS)
                    # af_b[i] = sum_d xv(i,d)*sh_v(d) -> xft cols 32:48
                    nc.vector.tensor_reduce(
                        out=xft4[:, :, 32:48],
                        in_=xft4[:, :, 96:144].rearrange(
                            "p s (i d) -> p s i d", d=3),
                        axis=mybir.AxisListType.X, op=OP.add)

                    wsb4 = wsb[:].rearrange("p (s q) -> p s q", q=WNUMEL)
                    # products
                    pa = sp.tile([128, 4 * 1536], BF16, tag="pa")
                    nc.vector.tensor_tensor(
                        out=pa[:].rearrange("p (s o j) -> p s o j", o=32, j=48),
                        in0=wsb4[:, :, 0:1536].rearrange(
                            "p s (o j) -> p s o j", j=48),
                        in1=xft4[:, :, 0:48].unsqueeze(2)
                            .to_broadcast([128, 4, 32, 48]),
                        op=OP.mult)
                    pb = sp.tile([128, 4 * 512], BF16, tag="pb")
                    nc.vector.tensor_tensor(
                        out=pb[:].rearrange("p (s o i) -> p s o i", o=16, i=32),
                        in0=wsb4[:, :, 1536:2048].rearrange(
                            "p s (o i) -> p s o i", i=32),
                        in1=xft4[:, :, 144:176].unsqueeze(2)
                            .to_broadcast([128, 4, 16, 32]),
                        op=OP.mult)
                    pc = sp.tile([128, 4 * 768], BF16, tag="pc")
                    pcv = pc[:].rearrange("p (s d q) -> p s d q", d=3, q=256)
                    for dd in range(3):
                        nc.vector.tensor_tensor(
                            out=pcv[:, :, dd, :].rearrange(
                                "p s (o i) -> p s o i", i=16),
                            in0=wsb4[:, :, 2048:2304].rearrange(
                                "p s (o i) -> p s o i", i=16),
                            in1=xft4[:, :, 48 + dd * 16:48 + (dd + 1) * 16]
                                .unsqueeze(2).to_broadcast([128, 4, 16, 16]),
                            op=OP.mult)

                    m_t = sp.tile([128, 4 * FDIM], BF16, tag="m")
                    m4 = m_t[:].rearrange("p (s f) -> p s f", f=FDIM)

                    # A tree: 48 -> 24 -> 12 -> 6 -> 3 -> reduce3
                    pa4 = pa[:].rearrange("p (s o j) -> p s o j", o=32, j=48)
                    ta1 = sp.tile([128, 4 * 768], BF16, tag="ta1")
                    t1v = ta1[:].rearrange("p (s o j) -> p s o j", o=32, j=24)
                    nc.vector.tensor_tensor(out=t1v, in0=pa4[:, :, :, 0:24],
                                            in1=pa4[:, :, :, 24:48], op=OP.add)
                    ta2 = sp.tile([128, 4 * 384], BF16, tag="ta2")
                    t2v = ta2[:].rearrange("p (s o j) -> p s o j", o=32, j=12)
                    nc.vector.tensor_tensor(out=t2v, in0=t1v[:, :, :, 0:12],
                                            in1=t1v[:, :, :, 12:24], op=OP.add)
                    ta3 = sp.tile([128, 4 * 192], BF16, tag="ta3")
                    t3v = ta3[:].rearrange("p (s o j) -> p s o j", o=32, j=6)
                    nc.vector.tensor_tensor(out=t3v, in0=t2v[:, :, :, 0:6],
                                            in1=t2v[:, :, :, 6:12], op=OP.add)
                    ta4 = sp.tile([128, 4 * 96], BF16, tag="ta4")
                    t4v = ta4[:].rearrange("p (s o j) -> p s o j", o=32, j=3)
                    nc.vector.tensor_tensor(out=t4v, in0=t3v[:, :, :, 0:3],
                                            in1=t3v[:, :, :, 3:6], op=OP.add)
                    nc.vector.tensor_reduce(
                        out=m4[:, :, 0:32], in_=t4v,
                        axis=mybir.AxisListType.X, op=OP.add)

                    # B tree: 32 -> 16 -> 8 -> 4 -> 2 -> add
                    pb4 = pb[:].rearrange("p (s o i) -> p s o i", o=16, i=32)
                    tb1 = sp.tile([128, 4 * 256], BF16, tag="tb1")
                    b1v = tb1[:].rearrange("p (s o i) -> p s o i", o=16, i=16)
                    nc.vector.tensor_tensor(out=b1v, in0=pb4[:, :, :, 0:16],
                                            in1=pb4[:, :, :, 16:32], op=OP.add)
                    tb2 = sp.tile([128, 4 * 128], BF16, tag="tb2")
                    b2v = tb2[:].rearrange("p (s o i) -> p s o i", o=16, i=8)
                    nc.vector.tensor_tensor(out=b2v, in0=b1v[:, :, :, 0:8],
                                            in1=b1v[:, :, :, 8:16], op=OP.add)
                    tb3 = sp.tile([128, 4 * 64], BF16, tag="tb3")
                    b3v = tb3[:].rearrange("p (s o i) -> p s o i", o=16, i=4)
                    nc.vector.tensor_tensor(out=b3v, in0=b2v[:, :, :, 0:4],
                                            in1=b2v[:, :, :, 4:8], op=OP.add)
                    tb4 = sp.tile([128, 4 * 32], BF16, tag="tb4")
                    b4v = tb4[:].rearrange("p (s o i) -> p s o i", o=16, i=2)
                    nc.vector.tensor_tensor(out=b4v, in0=b3v[:, :, :, 0:2],
                                            in1=b3v[:, :, :, 2:4], op=OP.add)
                    t2s = sp.tile([128, 4 * 16], BF16, tag="t2s")
                    t2s4 = t2s[:].rearrange("p (s o) -> p s o", o=16)
                    nc.vector.tensor_tensor(out=t2s4, in0=b4v[:, :, :, 0],
                                            in1=b4v[:, :, :, 1], op=OP.add)

                    # C tree: 16 -> 8 -> 4 -> 2 -> add  (groups (s,d,o))
                    pc4 = pc[:].rearrange("p (s g i) -> p s g i", g=48, i=16)
                    tc1 = sp.tile([128, 4 * 384], BF16, tag="tc1")
                    c1v = tc1[:].rearrange("p (s g i) -> p s g i", g=48, i=8)
                    nc.vector.tensor_tensor(out=c1v, in0=pc4[:, :, :, 0:8],
                                            in1=pc4[:, :, :, 8:16], op=OP.add)
                    tc2 = sp.tile([128, 4 * 192], BF16, tag="tc2")
                    c2v = tc2[:].rearrange("p (s g i) -> p s g i", g=48, i=4)
                    nc.vector.tensor_tensor(out=c2v, in0=c1v[:, :, :, 0:4],
                                            in1=c1v[:, :, :, 4:8], op=OP.add)
                    tc3 = sp.tile([128, 4 * 96], BF16, tag="tc3")
                    c3v = tc3[:].rearrange("p (s g i) -> p s g i", g=48, i=2)
                    nc.vector.tensor_tensor(out=c3v, in0=c2v[:, :, :, 0:2],
                                            in1=c2v[:, :, :, 2:4], op=OP.add)
                    v3 = sp.tile([128, 4 * 48], BF16, tag="v3")
                    v34 = v3[:].rearrange("p (s g) -> p s g", g=48)
                    nc.vector.tensor_tensor(out=v34, in0=c3v[:, :, :, 0],
                                            in1=c3v[:, :, :, 1], op=OP.add)

                    # m_v = t2 (x) sh_v  +  v3   (v3 already has sh_s folded)
                    mtmp = sp.tile([128, 4 * 48], BF16, tag="mtmp")
                    nc.vector.tensor_tensor(
                        out=mtmp[:].rearrange("p (s d o) -> p s d o", d=3, o=16),
                        in0=t2s4.unsqueeze(2).to_broadcast([128, 4, 3, 16]),
                        in1=shv_t[:].rearrange("p (s d o) -> p s d o", d=3, o=16),
                        op=OP.mult)
                    nc.vector.tensor_tensor(
                        out=m4[:, :, 32:80],
                        in0=mtmp[:].rearrange("p (s g) -> p s g", g=48),
                        in1=v34, op=OP.add)

                    # S-combine on PE, per sub-tile, into one PSUM bank
                    comb_ps = cpp.tile([128, 4 * FDIM], F32, tag="comb")
                    for c in range(4):
                        nc.tensor.matmul(
                            out=comb_ps[:, c * FDIM:(c + 1) * FDIM],
                            lhsT=s_t[:, c * 128:(c + 1) * 128],
                            rhs=m_t[:, c * FDIM:(c + 1) * FDIM],
                            start=True, stop=True)
                    comb_sb = sp.tile([128, 4 * FDIM], F32, tag="combsb")
                    nc.scalar.copy(comb_sb[:], comb_ps[:])
                    nc.sync.dma_start(
                        out=partials[r0:r0 + 512, :].rearrange(
                            "(c p) f -> p c f", c=4),
                        in_=comb_sb[:].rearrange("p (c f) -> p c f", c=4))

            # ---------------- pass 2: nodes ----------------
            with (
                tc.tile_pool(name="sb2", bufs=4) as s2,
                tc.tile_pool(name="ps2", bufs=2, space="PSUM") as p2p,
            ):
                for b in range(nb):
                    n0 = b * 128
                    g_t = s2.tile([128, 2], I32, tag="g")
                    nc.sync.dma_start(out=g_t[:], in_=g12[n0:n0 + 128, :])
                    wv_t = s2.tile([128, 2], F32, tag="wv")
                    nc.sync.dma_start(out=wv_t[:], in_=w12[n0:n0 + 128, :])
                    p1 = s2.tile([128, FDIM], F32, tag="p1")
                    nc.gpsimd.indirect_dma_start(
                        out=p1[:], out_offset=None, in_=partials[:],
                        in_offset=bass.IndirectOffsetOnAxis(
                            ap=g_t[:, 0:1], axis=0))
                    p2 = s2.tile([128, FDIM], F32, tag="p2")
                    nc.gpsimd.indirect_dma_start(
                        out=p2[:], out_offset=None, in_=partials[:],
                        in_offset=bass.IndirectOffsetOnAxis(
                            ap=g_t[:, 1:2], axis=0))
                    agg = s2.tile([128, FDIM], F32, tag="agg")
                    nc.scalar.activation(agg[:], p1[:], AF.Copy,
                                         scale=wv_t[:, 0:1])
                    agg2 = s2.tile([128, FDIM], F32, tag="agg2")
                    nc.scalar.activation(agg2[:], p2[:], AF.Copy,
                                         scale=wv_t[:, 1:2])
                    nc.vector.tensor_tensor(out=agg[:], in0=agg[:],
                                            in1=agg2[:], op=OP.add)

                    tp_ps = p2p.tile([FDIM, 128], F32, tag="tp")
                    nc.tensor.transpose(out=tp_ps[:], in_=agg[:],
                                        identity=ident_sb[:])
                    aggT = s2.tile([FDIM, 128], F32, tag="aggT")
                    nc.scalar.copy(aggT[:], tp_ps[:])

                    xsh_t = s2.tile([FDIM, 128], F32, tag="xsh")
                    nc.sync.dma_start(out=xsh_t[:], in_=xshT[:, n0:n0 + 128])
                    y_ps = p2p.tile([FDIM, 128], F32, tag="y")
                    nc.tensor.matmul(out=y_ps[:], lhsT=mout_sb[:], rhs=aggT[:],
                                     start=True, stop=False)
                    nc.tensor.matmul(out=y_ps[:], lhsT=msf_sb[:], rhs=xsh_t[:],
                                     start=False, stop=True)
                    y_sb = s2.tile([FDIM, 128], F32, tag="ysb")
                    nc.scalar.copy(y_sb[:], y_ps[:])
                    nc.sync.dma_start(out=yT[:, n0:n0 + 128], in_=y_sb[:])

    nc.compile()
    return nc


_PROGRAM_CACHE = {}


def _get_program(n_nodes, npc_pad, epad, num_cores):
    key = (n_nodes, npc_pad, epad, num_cores)
    if key not in _PROGRAM_CACHE:
        _PROGRAM_CACHE[key] = build_program(n_nodes, npc_pad, epad, num_cores)
    return _PROGRAM_CACHE[key]


def prepare_in_maps(x, edge_src, edge_dst, edge_sh, edge_rbf,
                    w1, b1, w2, b2, w3, b3, num_cores=NCORES):
    n = x.shape[0]
    npc = -(-n // num_cores)
    npc_pad = -(-npc // 128) * 128

    dst = np.asarray(edge_dst, np.int64)
    src = np.asarray(edge_src, np.int64)
    order = np.argsort(dst, kind="stable")
    dst_s = dst[order]
    src_s = src[order]
    sh_s = np.asarray(edge_sh, np.float32)[order]
    rbf_s = np.asarray(edge_rbf, np.float32)[order]

    bounds = np.searchsorted(dst_s, np.arange(num_cores + 1) * npc)
    counts = np.diff(bounds)
    epad = max(512, int(-(-counts.max() // 512) * 512))
    nsuper = epad // 512

    bf16 = mybir.dt.np(BF16)
    w1bh = np.concatenate([np.asarray(w1, np.float32),
                           np.asarray(b1, np.float32)[None, :]], 0).astype(bf16)
    w2bh = np.concatenate([np.asarray(w2, np.float32),
                           np.asarray(b2, np.float32)[None, :]], 0).astype(bf16)
    perm, scl = _build_w3_perm()
    w3p_f = np.concatenate(
        [np.asarray(w3, np.float32)[:, perm] * scl[None, :],
         (np.asarray(b3, np.float32)[perm] * scl)[None, :]], 0)
    w3ph = w3p_f.astype(bf16)
    identh = np.eye(128, dtype=np.float32)
    xf = np.asarray(x, np.float32)

    # x table: [xs 32 | zeros 16 | xv(d,i) 48 | xv(i,d) 48 | xs 32]
    xtbl = np.zeros((n, XCOLS), np.float32)
    xtbl[:, :MUL0] = xf[:, :MUL0]
    xv = xf[:, MUL0:].reshape(n, MUL1, 3)         # (i, d)
    xtbl[:, 48:96] = xv.transpose(0, 2, 1).reshape(n, 48)   # (d, i)
    xtbl[:, 96:144] = xf[:, MUL0:]                           # (i, d)
    xtbl[:, 144:176] = xf[:, :MUL0]

    in_maps = []
    meta = {"npc": npc, "npc_pad": npc_pad, "epad": epad, "n": n,
            "num_cores": num_cores}
    for c in range(num_cores):
        lo, hi = bounds[c], bounds[c + 1]
        ec = hi - lo
        csrc = np.zeros(epad, np.int32)
        csrc[:ec] = src_s[lo:hi]
        cdst = np.full(epad, -1, np.int64)
        cdst[:ec] = dst_s[lo:hi]
        csh = np.zeros((epad, 4), np.float32)
        csh[:ec] = sh_s[lo:hi]

        crbf = np.zeros((RBF + 1, epad), np.float32)
        crbf[:RBF, :ec] = rbf_s[lo:hi].T
        crbf[RBF, :] = 1.0
        crbf = crbf.astype(bf16)

        # per-edge scale pattern [epad, 144]:
        # [sh_s*32 | 0*16 | sh_s*48 | sh_v(i,d)*48]
        pat = np.zeros((epad, XCOLS), np.float32)
        pat[:, 0:32] = csh[:, 0:1]
        pat[:, 48:96] = csh[:, 0:1]
        pat[:, 96:144] = np.tile(csh[:, 1:4], (1, 16))
        pat[:ec, 144:176] = 1.0
        # shv repeated in (d, o) layout
        svr = np.repeat(csh[:, 1:4], 16, axis=1)  # [epad, 48]

        # S matrices per 128-edge tile
        d2 = cdst.reshape(-1, 128)
        S = (d2[:, :, None] == d2[:, None, :]) & (d2[:, :, None] >= 0)
        S = S.astype(np.float32)

        # reorder edge-major [epad] -> [nsuper, 128, 4(sub)]
        def to_g(a, width):
            a = a.reshape(nsuper, 4, 128, width)
            return a.transpose(0, 2, 1, 3).reshape(nsuper * 128, 4 * width)

        csrc_g = to_g(csrc.reshape(epad, 1), 1).astype(np.int32)
        pat_g = to_g(pat, XCOLS).astype(bf16)
        svr_g = to_g(svr, 48).astype(bf16)
        S_g = to_g(S.reshape(epad, 128), 128).astype(bf16)

        # node -> first/last edge rows (local), inv-degree folded weights
        nbase = c * npc
        nodes = np.arange(npc_pad, dtype=np.int64) + nbase
        first = np.searchsorted(dst_s[lo:hi], nodes, side="left")
        last = np.searchsorted(dst_s[lo:hi], nodes, side="right") - 1
        deg = (last - first + 1).astype(np.int64)
        has = deg > 0
        g = np.zeros((npc_pad, 2), np.int32)
        wv = np.zeros((npc_pad, 2), np.float32)
        g[has, 0] = first[has].astype(np.int32)
        g[has, 1] = last[has].astype(np.int32)
        inv = np.zeros(npc_pad, np.float32)
        inv[has] = 1.0 / deg[has]
        wv[has, 0] = inv[has]
        wv[has, 1] = ((first[has] // 128) != (last[has] // 128)) * inv[has]

        # self-path features in layout L: [s | v(d,o)]
        cxsh = np.zeros((FDIM, npc_pad), np.float32)
        sl = xf[nbase:min(nbase + npc, n)]
        m = sl.shape[0]
        cxsh[:MUL0, :m] = sl[:, :MUL0].T
        slv = sl[:, MUL0:].reshape(m, MUL1, 3)
        cxsh[MUL0:, :m] = slv.transpose(2, 1, 0).reshape(48, m)

        in_maps.append({
            "x_tbl": xtbl, "xshT": cxsh, "src_g": csrc_g, "shpat": pat_g,
            "shvrep": svr_g, "smat": S_g, "rbf17": crbf,
            "w1b": w1bh, "w2b": w2bh, "w3p": w3ph,
            "g12": g, "w12": wv, "ident": identh,
        })
    return in_maps, meta


def kernel(x, edge_src, edge_dst, edge_sh, edge_rbf,
           w1, b1, w2, b2, w3, b3, ws_self, wv_self, ws_out, wv_out,
           _trace=False):
    num_cores = NCORES
    in_maps, meta = prepare_in_maps(
        x, edge_src, edge_dst, edge_sh, edge_rbf, w1, b1, w2, b2, w3, b3,
        num_cores=num_cores)
    msf = _irrep_matrix_L(np.asarray(ws_self, np.float32),
                          np.asarray(wv_self, np.float32))
    mout = _irrep_matrix_L(np.asarray(ws_out, np.float32),
                           np.asarray(wv_out, np.float32))
    for m in in_maps:
        m["msf"] = msf
        m["mout"] = mout

    nc = _get_program(meta["n"], meta["npc_pad"], meta["epad"], num_cores)
    res = run_bass_kernel_spmd(nc, in_maps, list(range(num_cores)),
                               trace=_trace)

    n, npc = meta["n"], meta["npc"]
    y = np.empty((n, FDIM), np.float32)
    for c in range(num_cores):
        lo = c * npc
        hi = min(lo + npc, n)
        yTc = np.asarray(res.results[c]["yT"])[:, :hi - lo]
        # rows: [s(32) | v(d,o)] -> output cols [s | v(o,d)]
        y[lo:hi, :MUL0] = yTc[:MUL0].T
        v = yTc[MUL0:].reshape(3, MUL1, hi - lo)
        y[lo:hi, MUL0:] = v.transpose(2, 1, 0).reshape(hi - lo, 48)
    kernel._last_results = res
    return y


# revision 11
# speedup vs baseline: 1.0074x; 1.0074x over previous
"""Equivariant interaction block (gnn message passing) on 8 trn2 NeuronCores.

v2 strategy (per-core, edges dst-sorted and sharded by dst node range):
  pass 1 per 512-edge supertile:
    - radial MLP on PE (bias via ones-row), h2s [65, 512] bf16
    - per 128-edge sub-tile: W' = h2s_c @ w3p in 5 PSUM chunks (rotating
      banks), Scalar copies -> W'sb [128, (sub, 2304)] bf16
    - indirect-gather x rows (144-col table: xs | gap | xv(d,i) | xv(i,d))
    - ALL elementwise work batched at supertile granularity on DVE:
      one scale-mult (host-shipped shpat), one small reduce, 3 product
      ops (broadcast APs), binary add-trees, 2 assembly ops
    - host-shipped selection matrices S fuse the within-tile segment-sum
      on PE; partials written per supertile
  pass 2 per 128-node block: 2 indirect gathers of partials, weighted
    combine (host-folded inverse degree), PE transpose + fused irrep
    linear (f32), write yT.
"""

import os
import sys

import numpy as np

for _p in ("/opt/trn_rl_repo", os.path.expanduser("~/.axon_site/_ro/trn_rl_repo")):
    if os.path.isdir(_p) and _p not in sys.path:
        sys.path.insert(0, _p)

import concourse.bacc as bacc
import concourse.bass as bass
import concourse.mybir as mybir
import concourse.tile as tile
from concourse.bass_utils import run_bass_kernel_spmd

F32 = mybir.dt.float32
BF16 = mybir.dt.bfloat16
I32 = mybir.dt.int32
AF = mybir.ActivationFunctionType
OP = mybir.AluOpType

MUL0, MUL1 = 32, 16
RBF, HID = 16, 64
O1 = MUL0 * MUL0
O2 = O1 + MUL0 * MUL1
O3 = O2 + MUL1 * MUL1
WNUMEL = O3 + MUL1 * MUL0  # 2304
C_PATH = float(1.0 / np.sqrt(np.float32(MUL0 + MUL1)))
C_110 = float(1.0 / np.sqrt(3.0))
NCORES = 8
FDIM = MUL0 + 3 * MUL1  # 80
XCOLS = 176  # xs*shs 32 | afb 16 | xv(d,i) 48 | xv(i,d) 48 | xs 32


def _build_w3_perm():
    """Permutation + scale taking reference w3 columns into our layout.

    W' columns (2304):
      A [0,1536):    q = o*48 + j      (o in 32, j in 48)
          j < 32 : path1  W1[i=j, o]   -> src i*32+o          scale C_PATH
          j >= 32: path4  W4[i=j-32,o] -> src O3 + i*32+o     scale C_PATH*C110
      B [1536,2048): q = 1536 + o*32 + i (o in 16, i in 32)
          path2 W2[i, o] -> src O1 + i*16 + o                 scale C_PATH
      C [2048,2304): q = 2048 + o*16 + i (o in 16, i in 16)
          path3 W3[i, o] -> src O2 + i*16 + o                 scale C_PATH
    """
    src = np.zeros(WNUMEL, np.int64)
    scl = np.zeros(WNUMEL, np.float32)
    for o in range(MUL0):
        for j in range(48):
            q = o * 48 + j
            if j < 32:
                src[q] = j * MUL0 + o
                scl[q] = C_PATH
            else:
                src[q] = O3 + (j - 32) * MUL0 + o
                scl[q] = C_PATH * C_110
    for o in range(MUL1):
        for i in range(MUL0):
            q = 1536 + o * 32 + i
            src[q] = O1 + i * MUL1 + o
            scl[q] = C_PATH
    for o in range(MUL1):
        for i in range(MUL1):
            q = 2048 + o * 16 + i
            src[q] = O2 + i * MUL1 + o
            scl[q] = C_PATH
    return src, scl


def _irrep_matrix_L(ws, wv):
    """[80,80] M in internal layout L = [s(32) | v(d,o): 32+d*16+o]."""
    M = np.zeros((FDIM, FDIM), np.float32)
    M[:MUL0, :MUL0] = ws
    for d in range(3):
        b = MUL0 + d * MUL1
        M[b:b + MUL1, b:b + MUL1] = wv
    return M


def build_program(n_nodes, npc_pad, epad, num_cores):
    nsuper = epad // 512
    nb = npc_pad // 128
    assert epad % 512 == 0 and npc_pad % 128 == 0

    nc = bacc.Bacc(
        "TRN2",
        target_bir_lowering=False,
        debug=False,
        enable_asserts=False,
        num_devices=num_cores,
    )

    x_tbl = nc.dram_tensor("x_tbl", [n_nodes, XCOLS], F32, kind="ExternalInput")
    xshT = nc.dram_tensor("xshT", [FDIM, npc_pad], F32, kind="ExternalInput")
    src_g = nc.dram_tensor("src_g", [nsuper * 128, 4], I32, kind="ExternalInput")
    shpat = nc.dram_tensor("shpat", [nsuper * 128, 4 * XCOLS], BF16,
                           kind="ExternalInput")
    shvrep = nc.dram_tensor("shvrep", [nsuper * 128, 4 * 48], BF16,
                            kind="ExternalInput")
    smat = nc.dram_tensor("smat", [nsuper * 128, 4 * 128], BF16,
                          kind="ExternalInput")
    rbf17 = nc.dram_tensor("rbf17", [RBF + 1, epad], BF16, kind="ExternalInput")
    w1b = nc.dram_tensor("w1b", [RBF + 1, HID], BF16, kind="ExternalInput")
    w2b = nc.dram_tensor("w2b", [HID + 1, HID], BF16, kind="ExternalInput")
    w3p = nc.dram_tensor("w3p", [HID + 1, WNUMEL], BF16, kind="ExternalInput")
    g12 = nc.dram_tensor("g12", [npc_pad, 2], I32, kind="ExternalInput")
    w12 = nc.dram_tensor("w12", [npc_pad, 2], F32, kind="ExternalInput")
    msf = nc.dram_tensor("msf", [FDIM, FDIM], F32, kind="ExternalInput")
    mout = nc.dram_tensor("mout", [FDIM, FDIM], F32, kind="ExternalInput")
    ident = nc.dram_tensor("ident", [128, 128], F32, kind="ExternalInput")

    yT = nc.dram_tensor("yT", [FDIM, npc_pad], F32, kind="ExternalOutput")
    partials = nc.dram_tensor("partials", [epad, FDIM], F32)

    with tile.TileContext(nc) as tc:
        with (
            nc.allow_low_precision(reason="bf16 per-edge messages, f32 agg"),
            tc.tile_pool(name="const", bufs=1) as cp,
        ):
            w1b_sb = cp.tile([RBF + 1, HID], BF16)
            nc.sync.dma_start(out=w1b_sb[:], in_=w1b[:])
            w2b_sb = cp.tile([HID + 1, HID], BF16)
            nc.sync.dma_start(out=w2b_sb[:], in_=w2b[:])
            w3p_sb = cp.tile([HID + 1, WNUMEL], BF16)
            nc.sync.dma_start(out=w3p_sb[:], in_=w3p[:])
            ident_sb = cp.tile([128, 128], F32)
            nc.sync.dma_start(out=ident_sb[:], in_=ident[:])
            msf_sb = cp.tile([FDIM, FDIM], F32)
            nc.sync.dma_start(out=msf_sb[:], in_=msf[:])
            mout_sb = cp.tile([FDIM, FDIM], F32)
            nc.sync.dma_start(out=mout_sb[:], in_=mout[:])

            # ---------------- pass 1: edges ----------------
            with (
                tc.tile_pool(name="sb", bufs=2) as sp,
                tc.tile_pool(name="wps", bufs=5, space="PSUM") as wpp,
                tc.tile_pool(name="mlp", bufs=2, space="PSUM") as mpp,
                tc.tile_pool(name="cmb", bufs=1, space="PSUM") as cpp,
            ):
                for s in range(nsuper):
                    r0 = s * 512
                    rbf_t = sp.tile([RBF + 1, 512], BF16, tag="rbf")
                    nc.sync.dma_start(out=rbf_t[:], in_=rbf17[:, r0:r0 + 512])
                    src_t = sp.tile([128, 4], I32, tag="src")
                    nc.sync.dma_start(out=src_t[:],
                                      in_=src_g[s * 128:(s + 1) * 128, :])
                    shp_t = sp.tile([128, 4 * XCOLS], BF16, tag="shp")
                    nc.sync.dma_start(out=shp_t[:],
                                      in_=shpat[s * 128:(s + 1) * 128, :])
                    shv_t = sp.tile([128, 4 * 48], BF16, tag="shv")
                    nc.sync.dma_start(out=shv_t[:],
                                      in_=shvrep[s * 128:(s + 1) * 128, :])
                    s_t = sp.tile([128, 4 * 128], BF16, tag="smat")
                    nc.sync.dma_start(out=s_t[:],
                                      in_=smat[s * 128:(s + 1) * 128, :])

                    # radial MLP
                    h1_ps = mpp.tile([HID, 512], F32, tag="mlp")
                    nc.tensor.matmul(out=h1_ps[:], lhsT=w1b_sb[:], rhs=rbf_t[:],
                                     start=True, stop=True)
                    h1s = sp.tile([HID + 1, 512], BF16, tag="h1s")
                    nc.scalar.activation(h1s[:HID, :], h1_ps[:], AF.Silu)
                    nc.gpsimd.memset(h1s[HID:HID + 1, :], 1.0)
                    h2_ps = mpp.tile([HID, 512], F32, tag="mlp")
                    nc.tensor.matmul(out=h2_ps[:], lhsT=w2b_sb[:], rhs=h1s[:],
                                     start=True, stop=True)
                    h2s = sp.tile([HID + 1, 512], BF16, tag="h2s")
                    nc.scalar.activation(h2s[:HID, :], h2_ps[:], AF.Silu)
                    nc.gpsimd.memset(h2s[HID:HID + 1, :], 1.0)

                    # gather + W' per sub-tile
                    xg = sp.tile([128, 4 * XCOLS], F32, tag="xg")
                    wsb = sp.tile([128, 4 * WNUMEL], BF16, tag="wsb")
                    for c in range(4):
                        nc.gpsimd.indirect_dma_start(
                            out=xg[:, c * XCOLS:(c + 1) * XCOLS],
                            out_offset=None, in_=x_tbl[:],
                            in_offset=bass.IndirectOffsetOnAxis(
                                ap=src_t[:, c:c + 1], axis=0))
                        lhs = h2s[:, c * 128:(c + 1) * 128]
                        for k in range(5):
                            q0 = k * 512
                            q1 = min(q0 + 512, WNUMEL)
                            w_ps = wpp.tile([128, 512], F32, tag="w")
                            nc.tensor.matmul(out=w_ps[:, :q1 - q0], lhsT=lhs,
                                             rhs=w3p_sb[:, q0:q1],
                                             start=True, stop=True)
                            nc.scalar.copy(
                                wsb[:, c * WNUMEL + q0:c * WNUMEL + q1],
                                w_ps[:, :q1 - q0])

                    # ---- supertile-batched DVE ----
                    # xft = xg * shpat  (f32 x bf16 -> bf16)
                    xft = sp.tile([128, 4 * XCOLS], BF16, tag="xft")
                    nc.vector.tensor_tensor(out=xft[:], in0=xg[:], in1=shp_t[:],
                                            op=OP.mult)
                    xft4 = xft[:].rearrange("p (s f) -> p s f", f=XCOLS)
                    # af_b[i] = sum_d xv(i,d)*sh_v(d) -> xft cols 32:48
                    nc.vector.tensor_reduce(
                        out=xft4[:, :, 32:48],
                        in_=xft4[:, :, 96:144].rearrange(
                            "p s (i d) -> p s i d", d=3),
                        axis=mybir.AxisListType.X, op=OP.add)

                    wsb4 = wsb[:].rearrange("p (s q) -> p s q", q=WNUMEL)
                    # products
                    pa = sp.tile([128, 4 * 1536], BF16, tag="pa")
                    nc.vector.tensor_tensor(
                        out=pa[:].rearrange("p (s o j) -> p s o j", o=32, j=48),
                        in0=wsb4[:, :, 0:1536].rearrange(
                            "p s (o j) -> p s o j", j=48),
                        in1=xft4[:, :, 0:48].unsqueeze(2)
                            .to_broadcast([128, 4, 32, 48]),
                        op=OP.mult)
                    pb = sp.tile([128, 4 * 512], BF16, tag="pb")
                    nc.vector.tensor_tensor(
                        out=pb[:].rearrange("p (s o i) -> p s o i", o=16, i=32),
                        in0=wsb4[:, :, 1536:2048].rearrange(
                            "p s (o i) -> p s o i", i=32),
                        in1=xft4[:, :, 144:176].unsqueeze(2)
                            .to_broadcast([128, 4, 16, 32]),
                        op=OP.mult)
                    pc = sp.tile([128, 4 * 768], BF16, tag="pc")
                    for c in range(4):
                        nc.vector.tensor_tensor(
                            out=pc[:, c * 768:(c + 1) * 768].rearrange(
                                "p (d o i) -> p d o i", d=3, i=16),
                            in0=wsb[:, c * WNUMEL + 2048:c * WNUMEL + 2304]
                                .rearrange("p (o i) -> p o i", i=16)
                                .unsqueeze(1).to_broadcast([128, 3, 16, 16]),
                            in1=xft[:, c * XCOLS + 48:c * XCOLS + 96]
                                .rearrange("p (d i) -> p d i", i=16)
                                .unsqueeze(2).to_broadcast([128, 3, 16, 16]),
                            op=OP.mult)

                    m_t = sp.tile([128, 4 * FDIM], BF16, tag="m")
                    m4 = m_t[:].rearrange("p (s f) -> p s f", f=FDIM)

                    # A tree: 48 -> 24 -> 12 -> 6 -> 3 -> reduce3
                    pa4 = pa[:].rearrange("p (s o j) -> p s o j", o=32, j=48)
                    ta1 = sp.tile([128, 4 * 768], BF16, tag="ta1")
                    t1v = ta1[:].rearrange("p (s o j) -> p s o j", o=32, j=24)
                    nc.vector.tensor_tensor(out=t1v, in0=pa4[:, :, :, 0:24],
                                            in1=pa4[:, :, :, 24:48], op=OP.add)
                    ta2 = sp.tile([128, 4 * 384], BF16, tag="ta2")
                    t2v = ta2[:].rearrange("p (s o j) -> p s o j", o=32, j=12)
                    nc.vector.tensor_tensor(out=t2v, in0=t1v[:, :, :, 0:12],
                                            in1=t1v[:, :, :, 12:24], op=OP.add)
                    ta3 = sp.tile([128, 4 * 192], BF16, tag="ta3")
                    t3v = ta3[:].rearrange("p (s o j) -> p s o j", o=32, j=6)
                    nc.vector.tensor_tensor(out=t3v, in0=t2v[:, :, :, 0:6],
                                            in1=t2v[:, :, :, 6:12], op=OP.add)
                    ta4 = sp.tile([128, 4 * 96], BF16, tag="ta4")
                    t4v = ta4[:].rearrange("p (s o j) -> p s o j", o=32, j=3)
                    nc.vector.tensor_tensor(out=t4v, in0=t3v[:, :, :, 0:3],
                                            in1=t3v[:, :, :, 3:6], op=OP.add)
                    nc.vector.tensor_reduce(
                        out=m4[:, :, 0:32], in_=t4v,
                        axis=mybir.AxisListType.X, op=OP.add)

                    # B tree: 32 -> 16 -> 8 -> 4 -> 2 -> add
                    pb4 = pb[:].rearrange("p (s o i) -> p s o i", o=16, i=32)
                    tb1 = sp.tile([128, 4 * 256], BF16, tag="tb1")
                    b1v = tb1[:].rearrange("p (s o i) -> p s o i", o=16, i=16)
                    nc.vector.tensor_tensor(out=b1v, in0=pb4[:, :, :, 0:16],
                                            in1=pb4[:, :, :, 16:32], op=OP.add)
                    tb2 = sp.tile([128, 4 * 128], BF16, tag="tb2")
                    b2v = tb2[:].rearrange("p (s o i) -> p s o i", o=16, i=8)
                    nc.vector.tensor_tensor(out=b2v, in0=b1v[:, :, :, 0:8],
                                            in1=b1v[:, :, :, 8:16], op=OP.add)
                    tb3 = sp.tile([128, 4 * 64], BF16, tag="tb3")
                    b3v = tb3[:].rearrange("p (s o i) -> p s o i", o=16, i=4)
                    nc.vector.tensor_tensor(out=b3v, in0=b2v[:, :, :, 0:4],
                                            in1=b2v[:, :, :, 4:8], op=OP.add)
                    tb4 = sp.tile([128, 4 * 32], BF16, tag="tb4")
                    b4v = tb4[:].rearrange("p (s o i) -> p s o i", o=16, i=2)
                    nc.vector.tensor_tensor(out=b4v, in0=b3v[:, :, :, 0:2],
                                            in1=b3v[:, :, :, 2:4], op=OP.add)
                    t2s = sp.tile([128, 4 * 16], BF16, tag="t2s")
                    t2s4 = t2s[:].rearrange("p (s o) -> p s o", o=16)
                    nc.vector.tensor_tensor(out=t2s4, in0=b4v[:, :, :, 0],
                                            in1=b4v[:, :, :, 1], op=OP.add)

                    # C tree: 16 -> 8 -> 4 -> 2 -> add  (groups (s,d,o))
                    pc4 = pc[:].rearrange("p (s g i) -> p s g i", g=48, i=16)
                    tc1 = sp.tile([128, 4 * 384], BF16, tag="tc1")
                    c1v = tc1[:].rearrange("p (s g i) -> p s g i", g=48, i=8)
                    nc.vector.tensor_tensor(out=c1v, in0=pc4[:, :, :, 0:8],
                                            in1=pc4[:, :, :, 8:16], op=OP.add)
                    tc2 = sp.tile([128, 4 * 192], BF16, tag="tc2")
                    c2v = tc2[:].rearrange("p (s g i) -> p s g i", g=48, i=4)
                    nc.vector.tensor_tensor(out=c2v, in0=c1v[:, :, :, 0:4],
                                            in1=c1v[:, :, :, 4:8], op=OP.add)
                    tc3 = sp.tile([128, 4 * 96], BF16, tag="tc3")
                    c3v = tc3[:].rearrange("p (s g i) -> p s g i", g=48, i=2)
                    nc.vector.tensor_tensor(out=c3v, in0=c2v[:, :, :, 0:2],
                                            in1=c2v[:, :, :, 2:4], op=OP.add)
                    v3 = sp.tile([128, 4 * 48], BF16, tag="v3")
                    v34 = v3[:].rearrange("p (s g) -> p s g", g=48)
                    nc.vector.tensor_tensor(out=v34, in0=c3v[:, :, :, 0],
                                            in1=c3v[:, :, :, 1], op=OP.add)

                    # m_v = t2 (x) sh_v  +  v3   (v3 already has sh_s folded)
                    mtmp = sp.tile([128, 4 * 48], BF16, tag="mtmp")
                    nc.vector.tensor_tensor(
                        out=mtmp[:].rearrange("p (s d o) -> p s d o", d=3, o=16),
                        in0=t2s4.unsqueeze(2).to_broadcast([128, 4, 3, 16]),
                        in1=shv_t[:].rearrange("p (s d o) -> p s d o", d=3, o=16),
                        op=OP.mult)
                    nc.vector.tensor_tensor(
                        out=m4[:, :, 32:80],
                        in0=mtmp[:].rearrange("p (s g) -> p s g", g=48),
                        in1=v34, op=OP.add)

                    # S-combine on PE, per sub-tile, into one PSUM bank
                    comb_ps = cpp.tile([128, 4 * FDIM], F32, tag="comb")
                    for c in range(4):
                        nc.tensor.matmul(
                            out=comb_ps[:, c * FDIM:(c + 1) * FDIM],
                            lhsT=s_t[:, c * 128:(c + 1) * 128],
                            rhs=m_t[:, c * FDIM:(c + 1) * FDIM],
                            start=True, stop=True)
                    comb_sb = sp.tile([128, 4 * FDIM], F32, tag="combsb")
                    nc.scalar.copy(comb_sb[:], comb_ps[:])
                    nc.sync.dma_start(
                        out=partials[r0:r0 + 512, :].rearrange(
                            "(c p) f -> p c f", c=4),
                        in_=comb_sb[:].rearrange("p (c f) -> p c f", c=4))

            # ---------------- pass 2: nodes ----------------
            with (
                tc.tile_pool(name="sb2", bufs=3) as s2,
                tc.tile_pool(name="ps2", bufs=2, space="PSUM") as p2p,
            ):
                for b in range(nb):
                    n0 = b * 128
                    g_t = s2.tile([128, 2], I32, tag="g")
                    nc.sync.dma_start(out=g_t[:], in_=g12[n0:n0 + 128, :])
                    wv_t = s2.tile([128, 2], F32, tag="wv")
                    nc.sync.dma_start(out=wv_t[:], in_=w12[n0:n0 + 128, :])
                    p1 = s2.tile([128, FDIM], F32, tag="p1")
                    nc.gpsimd.indirect_dma_start(
                        out=p1[:], out_offset=None, in_=partials[:],
                        in_offset=bass.IndirectOffsetOnAxis(
                            ap=g_t[:, 0:1], axis=0))
                    p2 = s2.tile([128, FDIM], F32, tag="p2")
                    nc.gpsimd.indirect_dma_start(
                        out=p2[:], out_offset=None, in_=partials[:],
                        in_offset=bass.IndirectOffsetOnAxis(
                            ap=g_t[:, 1:2], axis=0))
                    agg = s2.tile([128, FDIM], F32, tag="agg")
                    nc.vector.tensor_scalar(
                        out=agg[:], in0=p1[:], scalar1=wv_t[:, 0:1],
                        scalar2=None, op0=OP.mult)
                    agg2 = s2.tile([128, FDIM], F32, tag="agg2")
                    nc.vector.tensor_scalar(
                        out=agg2[:], in0=p2[:], scalar1=wv_t[:, 1:2],
                        scalar2=None, op0=OP.mult)
                    nc.vector.tensor_tensor(out=agg[:], in0=agg[:],
                                            in1=agg2[:], op=OP.add)

                    tp_ps = p2p.tile([FDIM, 128], F32, tag="tp")
                    nc.tensor.transpose(out=tp_ps[:], in_=agg[:],
                                        identity=ident_sb[:])
                    aggT = s2.tile([FDIM, 128], F32, tag="aggT")
                    nc.scalar.copy(aggT[:], tp_ps[:])

                    xsh_t = s2.tile([FDIM, 128], F32, tag="xsh")
                    nc.sync.dma_start(out=xsh_t[:], in_=xshT[:, n0:n0 + 128])
                    y_ps = p2p.tile([FDIM, 128], F32, tag="y")
                    nc.tensor.matmul(out=y_ps[:], lhsT=mout_sb[:], rhs=aggT[:],
                                     start=True, stop=False)
                    nc.tensor.matmul(out=y_ps[:], lhsT=msf_sb[:], rhs=xsh_t[:],
                                     start=False, stop=True)
                    y_sb = s2.tile([FDIM, 128], F32, tag="ysb")
                    nc.scalar.copy(y_sb[:], y_ps[:])
                    nc.sync.dma_start(out=yT[:, n0:n0 + 128], in_=y_sb[:])

    nc.compile()
    return nc


_PROGRAM_CACHE = {}


def _get_program(n_nodes, npc_pad, epad, num_cores):
    key = (n_nodes, npc_pad, epad, num_cores)
    if key not in _PROGRAM_CACHE:
        _PROGRAM_CACHE[key] = build_program(n_nodes, npc_pad, epad, num_cores)
    return _PROGRAM_CACHE[key]


def prepare_in_maps(x, edge_src, edge_dst, edge_sh, edge_rbf,
                    w1, b1, w2, b2, w3, b3, num_cores=NCORES):
    n = x.shape[0]
    npc = -(-n // num_cores)
    npc_pad = -(-npc // 128) * 128

    dst = np.asarray(edge_dst, np.int64)
    src = np.asarray(edge_src, np.int64)
    order = np.argsort(dst, kind="stable")
    dst_s = dst[order]
    src_s = src[order]
    sh_s = np.asarray(edge_sh, np.float32)[order]
    rbf_s = np.asarray(edge_rbf, np.float32)[order]

    bounds = np.searchsorted(dst_s, np.arange(num_cores + 1) * npc)
    counts = np.diff(bounds)
    epad = max(512, int(-(-counts.max() // 512) * 512))
    nsuper = epad // 512

    bf16 = mybir.dt.np(BF16)
    w1bh = np.concatenate([np.asarray(w1, np.float32),
                           np.asarray(b1, np.float32)[None, :]], 0).astype(bf16)
    w2bh = np.concatenate([np.asarray(w2, np.float32),
                           np.asarray(b2, np.float32)[None, :]], 0).astype(bf16)
    perm, scl = _build_w3_perm()
    w3p_f = np.concatenate(
        [np.asarray(w3, np.float32)[:, perm] * scl[None, :],
         (np.asarray(b3, np.float32)[perm] * scl)[None, :]], 0)
    w3ph = w3p_f.astype(bf16)
    identh = np.eye(128, dtype=np.float32)
    xf = np.asarray(x, np.float32)

    # x table: [xs 32 | zeros 16 | xv(d,i) 48 | xv(i,d) 48 | xs 32]
    xtbl = np.zeros((n, XCOLS), np.float32)
    xtbl[:, :MUL0] = xf[:, :MUL0]
    xv = xf[:, MUL0:].reshape(n, MUL1, 3)         # (i, d)
    xtbl[:, 48:96] = xv.transpose(0, 2, 1).reshape(n, 48)   # (d, i)
    xtbl[:, 96:144] = xf[:, MUL0:]                           # (i, d)
    xtbl[:, 144:176] = xf[:, :MUL0]

    in_maps = []
    meta = {"npc": npc, "npc_pad": npc_pad, "epad": epad, "n": n,
            "num_cores": num_cores}
    for c in range(num_cores):
        lo, hi = bounds[c], bounds[c + 1]
        ec = hi - lo
        csrc = np.zeros(epad, np.int32)
        csrc[:ec] = src_s[lo:hi]
        cdst = np.full(epad, -1, np.int64)
        cdst[:ec] = dst_s[lo:hi]
        csh = np.zeros((epad, 4), np.float32)
        csh[:ec] = sh_s[lo:hi]

        crbf = np.zeros((RBF + 1, epad), np.float32)
        crbf[:RBF, :ec] = rbf_s[lo:hi].T
        crbf[RBF, :] = 1.0
        crbf = crbf.astype(bf16)

        # per-edge scale pattern [epad, 144]:
        # [sh_s*32 | 0*16 | sh_s*48 | sh_v(i,d)*48]
        pat = np.zeros((epad, XCOLS), np.float32)
        pat[:, 0:32] = csh[:, 0:1]
        pat[:, 48:96] = csh[:, 0:1]
        pat[:, 96:144] = np.tile(csh[:, 1:4], (1, 16))
        pat[:ec, 144:176] = 1.0
        # shv repeated in (d, o) layout
        svr = np.repeat(csh[:, 1:4], 16, axis=1)  # [epad, 48]

        # S matrices per 128-edge tile
        d2 = cdst.reshape(-1, 128)
        S = (d2[:, :, None] == d2[:, None, :]) & (d2[:, :, None] >= 0)
        S = S.astype(np.float32)

        # reorder edge-major [epad] -> [nsuper, 128, 4(sub)]
        def to_g(a, width):
            a = a.reshape(nsuper, 4, 128, width)
            return a.transpose(0, 2, 1, 3).reshape(nsuper * 128, 4 * width)

        csrc_g = to_g(csrc.reshape(epad, 1), 1).astype(np.int32)
        pat_g = to_g(pat, XCOLS).astype(bf16)
        svr_g = to_g(svr, 48).astype(bf16)
        S_g = to_g(S.reshape(epad, 128), 128).astype(bf16)

        # node -> first/last edge rows (local), inv-degree folded weights
        nbase = c * npc
        nodes = np.arange(npc_pad, dtype=np.int64) + nbase
        first = np.searchsorted(dst_s[lo:hi], nodes, side="left")
        last = np.searchsorted(dst_s[lo:hi], nodes, side="right") - 1
        deg = (last - first + 1).astype(np.int64)
        has = deg > 0
        g = np.zeros((npc_pad, 2), np.int32)
        wv = np.zeros((npc_pad, 2), np.float32)
        g[has, 0] = first[has].astype(np.int32)
        g[has, 1] = last[has].astype(np.int32)
        inv = np.zeros(npc_pad, np.float32)
        inv[has] = 1.0 / deg[has]
        wv[has, 0] = inv[has]
        wv[has, 1] = ((first[has] // 128) != (last[has] // 128)) * inv[has]

        # self-path features in layout L: [s | v(d,o)]
        cxsh = np.zeros((FDIM, npc_pad), np.float32)
        sl = xf[nbase:min(nbase + npc, n)]
        m = sl.shape[0]
        cxsh[:MUL0, :m] = sl[:, :MUL0].T
        slv = sl[:, MUL0:].reshape(m, MUL1, 3)
        cxsh[MUL0:, :m] = slv.transpose(2, 1, 0).reshape(48, m)

        in_maps.append({
            "x_tbl": xtbl, "xshT": cxsh, "src_g": csrc_g, "shpat": pat_g,
            "shvrep": svr_g, "smat": S_g, "rbf17": crbf,
            "w1b": w1bh, "w2b": w2bh, "w3p": w3ph,
            "g12": g, "w12": wv, "ident": identh,
        })
    return in_maps, meta


def kernel(x, edge_src, edge_dst, edge_sh, edge_rbf,
           w1, b1, w2, b2, w3, b3, ws_self, wv_self, ws_out, wv_out,
           _trace=False):
    num_cores = NCORES
    in_maps, meta = prepare_in_maps(
        x, edge_src, edge_dst, edge_sh, edge_rbf, w1, b1, w2, b2, w3, b3,
        num_cores=num_cores)
    msf = _irrep_matrix_L(np.asarray(ws_self, np.float32),
                          np.asarray(wv_self, np.float32))
    mout = _irrep_matrix_L(np.asarray(ws_out, np.float32),
                           np.asarray(wv_out, np.float32))
    for m in in_maps:
        m["msf"] = msf
        m["mout"] = mout

    nc = _get_program(meta["n"], meta["npc_pad"], meta["epad"], num_cores)
    res = run_bass_kernel_spmd(nc, in_maps, list(range(num_cores)),
                               trace=_trace)

    n, npc = meta["n"], meta["npc"]
    y = np.empty((n, FDIM), np.float32)
    for c in range(num_cores):
        lo = c * npc
        hi = min(lo + npc, n)
        yTc = np.asarray(res.results[c]["yT"])[:, :hi - lo]
        # rows: [s(32) | v(d,o)] -> output cols [s | v(o,d)]
        y[lo:hi, :MUL0] = yTc[:MUL0].T
        v = yTc[MUL0:].reshape(3, MUL1, hi - lo)
        y[lo:hi, MUL0:] = v.transpose(2, 1, 0).reshape(hi - lo, 48)
    kernel._last_results = res
    return y


# revision 12
# speedup vs baseline: 1.0234x; 1.0159x over previous
"""Equivariant interaction block (gnn message passing) on 8 trn2 NeuronCores.

v2 strategy (per-core, edges dst-sorted and sharded by dst node range):
  pass 1 per 512-edge supertile:
    - radial MLP on PE (bias via ones-row), h2s [65, 512] bf16
    - per 128-edge sub-tile: W' = h2s_c @ w3p in 5 PSUM chunks (rotating
      banks), Scalar copies -> W'sb [128, (sub, 2304)] bf16
    - indirect-gather x rows (144-col table: xs | gap | xv(d,i) | xv(i,d))
    - ALL elementwise work batched at supertile granularity on DVE:
      one scale-mult (host-shipped shpat), one small reduce, 3 product
      ops (broadcast APs), binary add-trees, 2 assembly ops
    - host-shipped selection matrices S fuse the within-tile segment-sum
      on PE; partials written per supertile
  pass 2 per 128-node block: 2 indirect gathers of partials, weighted
    combine (host-folded inverse degree), PE transpose + fused irrep
    linear (f32), write yT.
"""

import os
import sys

import numpy as np

for _p in ("/opt/trn_rl_repo", os.path.expanduser("~/.axon_site/_ro/trn_rl_repo")):
    if os.path.isdir(_p) and _p not in sys.path:
        sys.path.insert(0, _p)

import concourse.bacc as bacc
import concourse.bass as bass
import concourse.mybir as mybir
import concourse.tile as tile
from concourse.bass_utils import run_bass_kernel_spmd

F32 = mybir.dt.float32
BF16 = mybir.dt.bfloat16
I32 = mybir.dt.int32
AF = mybir.ActivationFunctionType
OP = mybir.AluOpType

MUL0, MUL1 = 32, 16
RBF, HID = 16, 64
O1 = MUL0 * MUL0
O2 = O1 + MUL0 * MUL1
O3 = O2 + MUL1 * MUL1
WNUMEL = O3 + MUL1 * MUL0  # 2304
C_PATH = float(1.0 / np.sqrt(np.float32(MUL0 + MUL1)))
C_110 = float(1.0 / np.sqrt(3.0))
NCORES = 8
FDIM = MUL0 + 3 * MUL1  # 80
XCOLS = 176  # xs*shs 32 | afb 16 | xv(d,i) 48 | xv(i,d) 48 | xs 32


def _build_w3_perm():
    """Permutation + scale taking reference w3 columns into our layout.

    W' columns (2304):
      A [0,1536):    q = o*48 + j      (o in 32, j in 48)
          j < 32 : path1  W1[i=j, o]   -> src i*32+o          scale C_PATH
          j >= 32: path4  W4[i=j-32,o] -> src O3 + i*32+o     scale C_PATH*C110
      B [1536,2048): q = 1536 + o*32 + i (o in 16, i in 32)
          path2 W2[i, o] -> src O1 + i*16 + o                 scale C_PATH
      C [2048,2304): q = 2048 + o*16 + i (o in 16, i in 16)
          path3 W3[i, o] -> src O2 + i*16 + o                 scale C_PATH
    """
    src = np.zeros(WNUMEL, np.int64)
    scl = np.zeros(WNUMEL, np.float32)
    for o in range(MUL0):
        for j in range(48):
            q = o * 48 + j
            if j < 32:
                src[q] = j * MUL0 + o
                scl[q] = C_PATH
            else:
                src[q] = O3 + (j - 32) * MUL0 + o
                scl[q] = C_PATH * C_110
    for o in range(MUL1):
        for i in range(MUL0):
            q = 1536 + o * 32 + i
            src[q] = O1 + i * MUL1 + o
            scl[q] = C_PATH
    for o in range(MUL1):
        for i in range(MUL1):
            q = 2048 + o * 16 + i
            src[q] = O2 + i * MUL1 + o
            scl[q] = C_PATH
    return src, scl


def _irrep_matrix_L(ws, wv):
    """[80,80] M in internal layout L = [s(32) | v(d,o): 32+d*16+o]."""
    M = np.zeros((FDIM, FDIM), np.float32)
    M[:MUL0, :MUL0] = ws
    for d in range(3):
        b = MUL0 + d * MUL1
        M[b:b + MUL1, b:b + MUL1] = wv
    return M


def build_program(n_nodes, npc_pad, epad, num_cores):
    nsuper = epad // 512
    nb = npc_pad // 128
    assert epad % 512 == 0 and npc_pad % 128 == 0

    nc = bacc.Bacc(
        "TRN2",
        target_bir_lowering=False,
        debug=False,
        enable_asserts=False,
        num_devices=num_cores,
    )

    x_tbl = nc.dram_tensor("x_tbl", [n_nodes, XCOLS], F32, kind="ExternalInput")
    xshT = nc.dram_tensor("xshT", [FDIM, npc_pad], F32, kind="ExternalInput")
    src_g = nc.dram_tensor("src_g", [nsuper * 128, 4], I32, kind="ExternalInput")
    shpat = nc.dram_tensor("shpat", [nsuper * 128, 4 * XCOLS], BF16,
                           kind="ExternalInput")
    shvrep = nc.dram_tensor("shvrep", [nsuper * 128, 4 * 48], BF16,
                            kind="ExternalInput")
    smat = nc.dram_tensor("smat", [nsuper * 128, 4 * 128], BF16,
                          kind="ExternalInput")
    rbf17 = nc.dram_tensor("rbf17", [RBF + 1, epad], BF16, kind="ExternalInput")
    w1b = nc.dram_tensor("w1b", [RBF + 1, HID], BF16, kind="ExternalInput")
    w2b = nc.dram_tensor("w2b", [HID + 1, HID], BF16, kind="ExternalInput")
    w3p = nc.dram_tensor("w3p", [HID + 1, WNUMEL], BF16, kind="ExternalInput")
    g12 = nc.dram_tensor("g12", [npc_pad, 2], I32, kind="ExternalInput")
    w12 = nc.dram_tensor("w12", [npc_pad, 2], F32, kind="ExternalInput")
    msf = nc.dram_tensor("msf", [FDIM, FDIM], F32, kind="ExternalInput")
    mout = nc.dram_tensor("mout", [FDIM, FDIM], F32, kind="ExternalInput")
    ident = nc.dram_tensor("ident", [128, 128], F32, kind="ExternalInput")

    yT = nc.dram_tensor("yT", [FDIM, npc_pad], F32, kind="ExternalOutput")
    partials = nc.dram_tensor("partials", [epad, FDIM], F32)

    with tile.TileContext(nc) as tc:
        with (
            nc.allow_low_precision(reason="bf16 per-edge messages, f32 agg"),
            tc.tile_pool(name="const", bufs=1) as cp,
        ):
            w1b_sb = cp.tile([RBF + 1, HID], BF16)
            nc.sync.dma_start(out=w1b_sb[:], in_=w1b[:])
            w2b_sb = cp.tile([HID + 1, HID], BF16)
            nc.sync.dma_start(out=w2b_sb[:], in_=w2b[:])
            w3p_sb = cp.tile([HID + 1, WNUMEL], BF16)
            nc.sync.dma_start(out=w3p_sb[:], in_=w3p[:])
            ident_sb = cp.tile([128, 128], F32)
            nc.sync.dma_start(out=ident_sb[:], in_=ident[:])
            msf_sb = cp.tile([FDIM, FDIM], F32)
            nc.sync.dma_start(out=msf_sb[:], in_=msf[:])
            mout_sb = cp.tile([FDIM, FDIM], F32)
            nc.sync.dma_start(out=mout_sb[:], in_=mout[:])

            # ---------------- pass 1: edges ----------------
            with (
                tc.tile_pool(name="sb", bufs=2) as sp,
                tc.tile_pool(name="wps", bufs=4, space="PSUM") as wpp,
                tc.tile_pool(name="mlp", bufs=2, space="PSUM") as mpp,
                tc.tile_pool(name="cmb", bufs=2, space="PSUM") as cpp,
            ):
                for s in range(nsuper):
                    r0 = s * 512
                    rbf_t = sp.tile([RBF + 1, 512], BF16, tag="rbf")
                    nc.sync.dma_start(out=rbf_t[:], in_=rbf17[:, r0:r0 + 512])
                    src_t = sp.tile([128, 4], I32, tag="src")
                    nc.sync.dma_start(out=src_t[:],
                                      in_=src_g[s * 128:(s + 1) * 128, :])
                    shp_t = sp.tile([128, 4 * XCOLS], BF16, tag="shp")
                    nc.sync.dma_start(out=shp_t[:],
                                      in_=shpat[s * 128:(s + 1) * 128, :])
                    shv_t = sp.tile([128, 4 * 48], BF16, tag="shv")
                    nc.sync.dma_start(out=shv_t[:],
                                      in_=shvrep[s * 128:(s + 1) * 128, :])
                    s_t = sp.tile([128, 4 * 128], BF16, tag="smat")
                    nc.sync.dma_start(out=s_t[:],
                                      in_=smat[s * 128:(s + 1) * 128, :])

                    # radial MLP
                    h1_ps = mpp.tile([HID, 512], F32, tag="mlp")
                    nc.tensor.matmul(out=h1_ps[:], lhsT=w1b_sb[:], rhs=rbf_t[:],
                                     start=True, stop=True)
                    h1s = sp.tile([HID + 1, 512], BF16, tag="h1s")
                    nc.scalar.activation(h1s[:HID, :], h1_ps[:], AF.Silu)
                    nc.gpsimd.memset(h1s[HID:HID + 1, :], 1.0)
                    h2_ps = mpp.tile([HID, 512], F32, tag="mlp")
                    nc.tensor.matmul(out=h2_ps[:], lhsT=w2b_sb[:], rhs=h1s[:],
                                     start=True, stop=True)
                    h2s = sp.tile([HID + 1, 512], BF16, tag="h2s")
                    nc.scalar.activation(h2s[:HID, :], h2_ps[:], AF.Silu)
                    nc.gpsimd.memset(h2s[HID:HID + 1, :], 1.0)

                    # gather + W' per sub-tile
                    xg = sp.tile([128, 4 * XCOLS], F32, tag="xg")
                    wsb = sp.tile([128, 4 * WNUMEL], BF16, tag="wsb")
                    for c in range(4):
                        nc.gpsimd.indirect_dma_start(
                            out=xg[:, c * XCOLS:(c + 1) * XCOLS],
                            out_offset=None, in_=x_tbl[:],
                            in_offset=bass.IndirectOffsetOnAxis(
                                ap=src_t[:, c:c + 1], axis=0))
                        lhs = h2s[:, c * 128:(c + 1) * 128]
                        for k in range(5):
                            q0 = k * 512
                            q1 = min(q0 + 512, WNUMEL)
                            w_ps = wpp.tile([128, 512], F32, tag="w")
                            nc.tensor.matmul(out=w_ps[:, :q1 - q0], lhsT=lhs,
                                             rhs=w3p_sb[:, q0:q1],
                                             start=True, stop=True)
                            nc.scalar.copy(
                                wsb[:, c * WNUMEL + q0:c * WNUMEL + q1],
                                w_ps[:, :q1 - q0])

                    # ---- supertile-batched DVE ----
                    # xft = xg * shpat  (f32 x bf16 -> bf16)
                    xft = sp.tile([128, 4 * XCOLS], BF16, tag="xft")
                    nc.vector.tensor_tensor(out=xft[:], in0=xg[:], in1=shp_t[:],
                                            op=OP.mult)
                    xft4 = xft[:].rearrange("p (s f) -> p s f", f=XCOLS)
                    # af_b[i] = sum_d xv(i,d)*sh_v(d) -> xft cols 32:48
                    nc.vector.tensor_reduce(
                        out=xft4[:, :, 32:48],
                        in_=xft4[:, :, 96:144].rearrange(
                            "p s (i d) -> p s i d", d=3),
                        axis=mybir.AxisListType.X, op=OP.add)

                    wsb4 = wsb[:].rearrange("p (s q) -> p s q", q=WNUMEL)
                    # products
                    pa = sp.tile([128, 4 * 1536], BF16, tag="pa")
                    nc.vector.tensor_tensor(
                        out=pa[:].rearrange("p (s o j) -> p s o j", o=32, j=48),
                        in0=wsb4[:, :, 0:1536].rearrange(
                            "p s (o j) -> p s o j", j=48),
                        in1=xft4[:, :, 0:48].unsqueeze(2)
                            .to_broadcast([128, 4, 32, 48]),
                        op=OP.mult)
                    pb = sp.tile([128, 4 * 512], BF16, tag="pb")
                    nc.vector.tensor_tensor(
                        out=pb[:].rearrange("p (s o i) -> p s o i", o=16, i=32),
                        in0=wsb4[:, :, 1536:2048].rearrange(
                            "p s (o i) -> p s o i", i=32),
                        in1=xft4[:, :, 144:176].unsqueeze(2)
                            .to_broadcast([128, 4, 16, 32]),
                        op=OP.mult)
                    pc = sp.tile([128, 4 * 768], BF16, tag="pc")
                    for c in range(4):
                        nc.vector.tensor_tensor(
                            out=pc[:, c * 768:(c + 1) * 768].rearrange(
                                "p (d o i) -> p d o i", d=3, i=16),
                            in0=wsb[:, c * WNUMEL + 2048:c * WNUMEL + 2304]
                                .rearrange("p (o i) -> p o i", i=16)
                                .unsqueeze(1).to_broadcast([128, 3, 16, 16]),
                            in1=xft[:, c * XCOLS + 48:c * XCOLS + 96]
                                .rearrange("p (d i) -> p d i", i=16)
                                .unsqueeze(2).to_broadcast([128, 3, 16, 16]),
                            op=OP.mult)

                    m_t = sp.tile([128, 4 * FDIM], BF16, tag="m")
                    m4 = m_t[:].rearrange("p (s f) -> p s f", f=FDIM)

                    # A tree: 48 -> 24 -> 12 -> 6 -> 3 -> reduce3
                    pa4 = pa[:].rearrange("p (s o j) -> p s o j", o=32, j=48)
                    ta1 = sp.tile([128, 4 * 768], BF16, tag="ta1")
                    t1v = ta1[:].rearrange("p (s o j) -> p s o j", o=32, j=24)
                    nc.vector.tensor_tensor(out=t1v, in0=pa4[:, :, :, 0:24],
                                            in1=pa4[:, :, :, 24:48], op=OP.add)
                    ta2 = sp.tile([128, 4 * 384], BF16, tag="ta2")
                    t2v = ta2[:].rearrange("p (s o j) -> p s o j", o=32, j=12)
                    nc.vector.tensor_tensor(out=t2v, in0=t1v[:, :, :, 0:12],
                                            in1=t1v[:, :, :, 12:24], op=OP.add)
                    ta3 = sp.tile([128, 4 * 192], BF16, tag="ta3")
                    t3v = ta3[:].rearrange("p (s o j) -> p s o j", o=32, j=6)
                    nc.vector.tensor_tensor(out=t3v, in0=t2v[:, :, :, 0:6],
                                            in1=t2v[:, :, :, 6:12], op=OP.add)
                    ta4 = sp.tile([128, 4 * 96], BF16, tag="ta4")
                    t4v = ta4[:].rearrange("p (s o j) -> p s o j", o=32, j=3)
                    nc.vector.tensor_tensor(out=t4v, in0=t3v[:, :, :, 0:3],
                                            in1=t3v[:, :, :, 3:6], op=OP.add)
                    nc.vector.tensor_reduce(
                        out=m4[:, :, 0:32], in_=t4v,
                        axis=mybir.AxisListType.X, op=OP.add)

                    # B tree: 32 -> 16 -> 8 -> 4 -> 2 -> add
                    pb4 = pb[:].rearrange("p (s o i) -> p s o i", o=16, i=32)
                    tb1 = sp.tile([128, 4 * 256], BF16, tag="tb1")
                    b1v = tb1[:].rearrange("p (s o i) -> p s o i", o=16, i=16)
                    nc.vector.tensor_tensor(out=b1v, in0=pb4[:, :, :, 0:16],
                                            in1=pb4[:, :, :, 16:32], op=OP.add)
                    tb2 = sp.tile([128, 4 * 128], BF16, tag="tb2")
                    b2v = tb2[:].rearrange("p (s o i) -> p s o i", o=16, i=8)
                    nc.vector.tensor_tensor(out=b2v, in0=b1v[:, :, :, 0:8],
                                            in1=b1v[:, :, :, 8:16], op=OP.add)
                    tb3 = sp.tile([128, 4 * 64], BF16, tag="tb3")
                    b3v = tb3[:].rearrange("p (s o i) -> p s o i", o=16, i=4)
                    nc.vector.tensor_tensor(out=b3v, in0=b2v[:, :, :, 0:4],
                                            in1=b2v[:, :, :, 4:8], op=OP.add)
                    tb4 = sp.tile([128, 4 * 32], BF16, tag="tb4")
                    b4v = tb4[:].rearrange("p (s o i) -> p s o i", o=16, i=2)
                    nc.vector.tensor_tensor(out=b4v, in0=b3v[:, :, :, 0:2],
                                            in1=b3v[:, :, :, 2:4], op=OP.add)
                    t2s = sp.tile([128, 4 * 16], BF16, tag="t2s")
                    t2s4 = t2s[:].rearrange("p (s o) -> p s o", o=16)
                    nc.vector.tensor_tensor(out=t2s4, in0=b4v[:, :, :, 0],
                                            in1=b4v[:, :, :, 1], op=OP.add)

                    # C tree: 16 -> 8 -> 4 -> 2 -> add  (groups (s,d,o))
                    pc4 = pc[:].rearrange("p (s g i) -> p s g i", g=48, i=16)
                    tc1 = sp.tile([128, 4 * 384], BF16, tag="tc1")
                    c1v = tc1[:].rearrange("p (s g i) -> p s g i", g=48, i=8)
                    nc.vector.tensor_tensor(out=c1v, in0=pc4[:, :, :, 0:8],
                                            in1=pc4[:, :, :, 8:16], op=OP.add)
                    tc2 = sp.tile([128, 4 * 192], BF16, tag="tc2")
                    c2v = tc2[:].rearrange("p (s g i) -> p s g i", g=48, i=4)
                    nc.vector.tensor_tensor(out=c2v, in0=c1v[:, :, :, 0:4],
                                            in1=c1v[:, :, :, 4:8], op=OP.add)
                    tc3 = sp.tile([128, 4 * 96], BF16, tag="tc3")
                    c3v = tc3[:].rearrange("p (s g i) -> p s g i", g=48, i=2)
                    nc.vector.tensor_tensor(out=c3v, in0=c2v[:, :, :, 0:2],
                                            in1=c2v[:, :, :, 2:4], op=OP.add)
                    v3 = sp.tile([128, 4 * 48], BF16, tag="v3")
                    v34 = v3[:].rearrange("p (s g) -> p s g", g=48)
                    nc.vector.tensor_tensor(out=v34, in0=c3v[:, :, :, 0],
                                            in1=c3v[:, :, :, 1], op=OP.add)

                    # m_v = t2 (x) sh_v  +  v3   (v3 already has sh_s folded)
                    mtmp = sp.tile([128, 4 * 48], BF16, tag="mtmp")
                    nc.vector.tensor_tensor(
                        out=mtmp[:].rearrange("p (s d o) -> p s d o", d=3, o=16),
                        in0=t2s4.unsqueeze(2).to_broadcast([128, 4, 3, 16]),
                        in1=shv_t[:].rearrange("p (s d o) -> p s d o", d=3, o=16),
                        op=OP.mult)
                    nc.vector.tensor_tensor(
                        out=m4[:, :, 32:80],
                        in0=mtmp[:].rearrange("p (s g) -> p s g", g=48),
                        in1=v34, op=OP.add)

                    # S-combine on PE, per sub-tile, into one PSUM bank
                    comb_ps = cpp.tile([128, 4 * FDIM], F32, tag="comb")
                    for c in range(4):
                        nc.tensor.matmul(
                            out=comb_ps[:, c * FDIM:(c + 1) * FDIM],
                            lhsT=s_t[:, c * 128:(c + 1) * 128],
                            rhs=m_t[:, c * FDIM:(c + 1) * FDIM],
                            start=True, stop=True)
                    comb_sb = sp.tile([128, 4 * FDIM], F32, tag="combsb")
                    nc.scalar.copy(comb_sb[:], comb_ps[:])
                    nc.sync.dma_start(
                        out=partials[r0:r0 + 512, :].rearrange(
                            "(c p) f -> p c f", c=4),
                        in_=comb_sb[:].rearrange("p (c f) -> p c f", c=4))

            # ---------------- pass 2: nodes ----------------
            with (
                tc.tile_pool(name="sb2", bufs=3) as s2,
                tc.tile_pool(name="ps2", bufs=2, space="PSUM") as p2p,
            ):
                for b in range(nb):
                    n0 = b * 128
                    g_t = s2.tile([128, 2], I32, tag="g")
                    nc.sync.dma_start(out=g_t[:], in_=g12[n0:n0 + 128, :])
                    wv_t = s2.tile([128, 2], F32, tag="wv")
                    nc.sync.dma_start(out=wv_t[:], in_=w12[n0:n0 + 128, :])
                    p1 = s2.tile([128, FDIM], F32, tag="p1")
                    nc.gpsimd.indirect_dma_start(
                        out=p1[:], out_offset=None, in_=partials[:],
                        in_offset=bass.IndirectOffsetOnAxis(
                            ap=g_t[:, 0:1], axis=0))
                    p2 = s2.tile([128, FDIM], F32, tag="p2")
                    nc.gpsimd.indirect_dma_start(
                        out=p2[:], out_offset=None, in_=partials[:],
                        in_offset=bass.IndirectOffsetOnAxis(
                            ap=g_t[:, 1:2], axis=0))
                    agg = s2.tile([128, FDIM], F32, tag="agg")
                    nc.vector.tensor_scalar(
                        out=agg[:], in0=p1[:], scalar1=wv_t[:, 0:1],
                        scalar2=None, op0=OP.mult)
                    agg2 = s2.tile([128, FDIM], F32, tag="agg2")
                    nc.vector.tensor_scalar(
                        out=agg2[:], in0=p2[:], scalar1=wv_t[:, 1:2],
                        scalar2=None, op0=OP.mult)
                    nc.vector.tensor_tensor(out=agg[:], in0=agg[:],
                                            in1=agg2[:], op=OP.add)

                    tp_ps = p2p.tile([FDIM, 128], F32, tag="tp")
                    nc.tensor.transpose(out=tp_ps[:], in_=agg[:],
                                        identity=ident_sb[:])
                    aggT = s2.tile([FDIM, 128], F32, tag="aggT")
                    nc.scalar.copy(aggT[:], tp_ps[:])

                    xsh_t = s2.tile([FDIM, 128], F32, tag="xsh")
                    nc.sync.dma_start(out=xsh_t[:], in_=xshT[:, n0:n0 + 128])
                    y_ps = p2p.tile([FDIM, 128], F32, tag="y")
                    nc.tensor.matmul(out=y_ps[:], lhsT=mout_sb[:], rhs=aggT[:],
                                     start=True, stop=False)
                    nc.tensor.matmul(out=y_ps[:], lhsT=msf_sb[:], rhs=xsh_t[:],
                                     start=False, stop=True)
                    y_sb = s2.tile([FDIM, 128], F32, tag="ysb")
                    nc.scalar.copy(y_sb[:], y_ps[:])
                    nc.sync.dma_start(out=yT[:, n0:n0 + 128], in_=y_sb[:])

    nc.compile()
    return nc


_PROGRAM_CACHE = {}


def _get_program(n_nodes, npc_pad, epad, num_cores):
    key = (n_nodes, npc_pad, epad, num_cores)
    if key not in _PROGRAM_CACHE:
        _PROGRAM_CACHE[key] = build_program(n_nodes, npc_pad, epad, num_cores)
    return _PROGRAM_CACHE[key]


def prepare_in_maps(x, edge_src, edge_dst, edge_sh, edge_rbf,
                    w1, b1, w2, b2, w3, b3, num_cores=NCORES):
    n = x.shape[0]
    npc = -(-n // num_cores)
    npc_pad = -(-npc // 128) * 128

    dst = np.asarray(edge_dst, np.int64)
    src = np.asarray(edge_src, np.int64)
    order = np.argsort(dst, kind="stable")
    dst_s = dst[order]
    src_s = src[order]
    sh_s = np.asarray(edge_sh, np.float32)[order]
    rbf_s = np.asarray(edge_rbf, np.float32)[order]

    bounds = np.searchsorted(dst_s, np.arange(num_cores + 1) * npc)
    counts = np.diff(bounds)
    epad = max(512, int(-(-counts.max() // 512) * 512))
    nsuper = epad // 512

    bf16 = mybir.dt.np(BF16)
    w1bh = np.concatenate([np.asarray(w1, np.float32),
                           np.asarray(b1, np.float32)[None, :]], 0).astype(bf16)
    w2bh = np.concatenate([np.asarray(w2, np.float32),
                           np.asarray(b2, np.float32)[None, :]], 0).astype(bf16)
    perm, scl = _build_w3_perm()
    w3p_f = np.concatenate(
        [np.asarray(w3, np.float32)[:, perm] * scl[None, :],
         (np.asarray(b3, np.float32)[perm] * scl)[None, :]], 0)
    w3ph = w3p_f.astype(bf16)
    identh = np.eye(128, dtype=np.float32)
    xf = np.asarray(x, np.float32)

    # x table: [xs 32 | zeros 16 | xv(d,i) 48 | xv(i,d) 48 | xs 32]
    xtbl = np.zeros((n, XCOLS), np.float32)
    xtbl[:, :MUL0] = xf[:, :MUL0]
    xv = xf[:, MUL0:].reshape(n, MUL1, 3)         # (i, d)
    xtbl[:, 48:96] = xv.transpose(0, 2, 1).reshape(n, 48)   # (d, i)
    xtbl[:, 96:144] = xf[:, MUL0:]                           # (i, d)
    xtbl[:, 144:176] = xf[:, :MUL0]

    in_maps = []
    meta = {"npc": npc, "npc_pad": npc_pad, "epad": epad, "n": n,
            "num_cores": num_cores}
    for c in range(num_cores):
        lo, hi = bounds[c], bounds[c + 1]
        ec = hi - lo
        csrc = np.zeros(epad, np.int32)
        csrc[:ec] = src_s[lo:hi]
        cdst = np.full(epad, -1, np.int64)
        cdst[:ec] = dst_s[lo:hi]
        csh = np.zeros((epad, 4), np.float32)
        csh[:ec] = sh_s[lo:hi]

        crbf = np.zeros((RBF + 1, epad), np.float32)
        crbf[:RBF, :ec] = rbf_s[lo:hi].T
        crbf[RBF, :] = 1.0
        crbf = crbf.astype(bf16)

        # per-edge scale pattern [epad, 144]:
        # [sh_s*32 | 0*16 | sh_s*48 | sh_v(i,d)*48]
        pat = np.zeros((epad, XCOLS), np.float32)
        pat[:, 0:32] = csh[:, 0:1]
        pat[:, 48:96] = csh[:, 0:1]
        pat[:, 96:144] = np.tile(csh[:, 1:4], (1, 16))
        pat[:ec, 144:176] = 1.0
        # shv repeated in (d, o) layout
        svr = np.repeat(csh[:, 1:4], 16, axis=1)  # [epad, 48]

        # S matrices per 128-edge tile
        d2 = cdst.reshape(-1, 128)
        S = (d2[:, :, None] == d2[:, None, :]) & (d2[:, :, None] >= 0)
        S = S.astype(np.float32)

        # reorder edge-major [epad] -> [nsuper, 128, 4(sub)]
        def to_g(a, width):
            a = a.reshape(nsuper, 4, 128, width)
            return a.transpose(0, 2, 1, 3).reshape(nsuper * 128, 4 * width)

        csrc_g = to_g(csrc.reshape(epad, 1), 1).astype(np.int32)
        pat_g = to_g(pat, XCOLS).astype(bf16)
        svr_g = to_g(svr, 48).astype(bf16)
        S_g = to_g(S.reshape(epad, 128), 128).astype(bf16)

        # node -> first/last edge rows (local), inv-degree folded weights
        nbase = c * npc
        nodes = np.arange(npc_pad, dtype=np.int64) + nbase
        first = np.searchsorted(dst_s[lo:hi], nodes, side="left")
        last = np.searchsorted(dst_s[lo:hi], nodes, side="right") - 1
        deg = (last - first + 1).astype(np.int64)
        has = deg > 0
        g = np.zeros((npc_pad, 2), np.int32)
        wv = np.zeros((npc_pad, 2), np.float32)
        g[has, 0] = first[has].astype(np.int32)
        g[has, 1] = last[has].astype(np.int32)
        inv = np.zeros(npc_pad, np.float32)
        inv[has] = 1.0 / deg[has]
        wv[has, 0] = inv[has]
        wv[has, 1] = ((first[has] // 128) != (last[has] // 128)) * inv[has]

        # self-path features in layout L: [s | v(d,o)]
        cxsh = np.zeros((FDIM, npc_pad), np.float32)
        sl = xf[nbase:min(nbase + npc, n)]
        m = sl.shape[0]
        cxsh[:MUL0, :m] = sl[:, :MUL0].T
        slv = sl[:, MUL0:].reshape(m, MUL1, 3)
        cxsh[MUL0:, :m] = slv.transpose(2, 1, 0).reshape(48, m)

        in_maps.append({
            "x_tbl": xtbl, "xshT": cxsh, "src_g": csrc_g, "shpat": pat_g,
            "shvrep": svr_g, "smat": S_g, "rbf17": crbf,
            "w1b": w1bh, "w2b": w2bh, "w3p": w3ph,
            "g12": g, "w12": wv, "ident": identh,
        })
    return in_maps, meta


def kernel(x, edge_src, edge_dst, edge_sh, edge_rbf,
           w1, b1, w2, b2, w3, b3, ws_self, wv_self, ws_out, wv_out,
           _trace=False):
    num_cores = NCORES
    in_maps, meta = prepare_in_maps(
        x, edge_src, edge_dst, edge_sh, edge_rbf, w1, b1, w2, b2, w3, b3,
        num_cores=num_cores)
    msf = _irrep_matrix_L(np.asarray(ws_self, np.float32),
                          np.asarray(wv_self, np.float32))
    mout = _irrep_matrix_L(np.asarray(ws_out, np.float32),
                           np.asarray(wv_out, np.float32))
    for m in in_maps:
        m["msf"] = msf
        m["mout"] = mout

    nc = _get_program(meta["n"], meta["npc_pad"], meta["epad"], num_cores)
    res = run_bass_kernel_spmd(nc, in_maps, list(range(num_cores)),
                               trace=_trace)

    n, npc = meta["n"], meta["npc"]
    y = np.empty((n, FDIM), np.float32)
    for c in range(num_cores):
        lo = c * npc
        hi = min(lo + npc, n)
        yTc = np.asarray(res.results[c]["yT"])[:, :hi - lo]
        # rows: [s(32) | v(d,o)] -> output cols [s | v(o,d)]
        y[lo:hi, :MUL0] = yTc[:MUL0].T
        v = yTc[MUL0:].reshape(3, MUL1, hi - lo)
        y[lo:hi, MUL0:] = v.transpose(2, 1, 0).reshape(hi - lo, 48)
    kernel._last_results = res
    return y


# revision 14
# speedup vs baseline: 1.0339x; 1.0103x over previous
"""Equivariant interaction block (gnn message passing) on 8 trn2 NeuronCores.

v2 strategy (per-core, edges dst-sorted and sharded by dst node range):
  pass 1 per 512-edge supertile:
    - radial MLP on PE (bias via ones-row), h2s [65, 512] bf16
    - per 128-edge sub-tile: W' = h2s_c @ w3p in 5 PSUM chunks (rotating
      banks), Scalar copies -> W'sb [128, (sub, 2304)] bf16
    - indirect-gather x rows (144-col table: xs | gap | xv(d,i) | xv(i,d))
    - ALL elementwise work batched at supertile granularity on DVE:
      one scale-mult (host-shipped shpat), one small reduce, 3 product
      ops (broadcast APs), binary add-trees, 2 assembly ops
    - host-shipped selection matrices S fuse the within-tile segment-sum
      on PE; partials written per supertile
  pass 2 per 128-node block: 2 indirect gathers of partials, weighted
    combine (host-folded inverse degree), PE transpose + fused irrep
    linear (f32), write yT.
"""

import os
import sys

import numpy as np

for _p in ("/opt/trn_rl_repo", os.path.expanduser("~/.axon_site/_ro/trn_rl_repo")):
    if os.path.isdir(_p) and _p not in sys.path:
        sys.path.insert(0, _p)

import concourse.bacc as bacc
import concourse.bass as bass
import concourse.mybir as mybir
import concourse.tile as tile
from concourse.bass_utils import run_bass_kernel_spmd

F32 = mybir.dt.float32
BF16 = mybir.dt.bfloat16
I32 = mybir.dt.int32
AF = mybir.ActivationFunctionType
OP = mybir.AluOpType

MUL0, MUL1 = 32, 16
RBF, HID = 16, 64
O1 = MUL0 * MUL0
O2 = O1 + MUL0 * MUL1
O3 = O2 + MUL1 * MUL1
WNUMEL = O3 + MUL1 * MUL0  # 2304
C_PATH = float(1.0 / np.sqrt(np.float32(MUL0 + MUL1)))
C_110 = float(1.0 / np.sqrt(3.0))
NCORES = 8
FDIM = MUL0 + 3 * MUL1  # 80
XCOLS = 176  # xs*shs 32 | afb 16 | xv(d,i) 48 | xv(i,d) 48 | xs 32


def _build_w3_perm():
    """Permutation + scale taking reference w3 columns into our layout.

    W' columns (2304):
      A [0,1536):    q = o*48 + j      (o in 32, j in 48)
          j < 32 : path1  W1[i=j, o]   -> src i*32+o          scale C_PATH
          j >= 32: path4  W4[i=j-32,o] -> src O3 + i*32+o     scale C_PATH*C110
      B [1536,2048): q = 1536 + o*32 + i (o in 16, i in 32)
          path2 W2[i, o] -> src O1 + i*16 + o                 scale C_PATH
      C [2048,2304): q = 2048 + o*16 + i (o in 16, i in 16)
          path3 W3[i, o] -> src O2 + i*16 + o                 scale C_PATH
    """
    src = np.zeros(WNUMEL, np.int64)
    scl = np.zeros(WNUMEL, np.float32)
    for o in range(MUL0):
        for j in range(48):
            q = o * 48 + j
            if j < 32:
                src[q] = j * MUL0 + o
                scl[q] = C_PATH
            else:
                src[q] = O3 + (j - 32) * MUL0 + o
                scl[q] = C_PATH * C_110
    for o in range(MUL1):
        for i in range(MUL0):
            q = 1536 + o * 32 + i
            src[q] = O1 + i * MUL1 + o
            scl[q] = C_PATH
    for o in range(MUL1):
        for i in range(MUL1):
            q = 2048 + o * 16 + i
            src[q] = O2 + i * MUL1 + o
            scl[q] = C_PATH
    return src, scl


def _irrep_matrix_L(ws, wv):
    """[80,80] M in internal layout L = [s(32) | v(d,o): 32+d*16+o]."""
    M = np.zeros((FDIM, FDIM), np.float32)
    M[:MUL0, :MUL0] = ws
    for d in range(3):
        b = MUL0 + d * MUL1
        M[b:b + MUL1, b:b + MUL1] = wv
    return M


def build_program(n_nodes, npc_pad, epad, num_cores):
    nsuper = epad // 512
    nb = npc_pad // 128
    assert epad % 512 == 0 and npc_pad % 128 == 0

    nc = bacc.Bacc(
        "TRN2",
        target_bir_lowering=False,
        debug=False,
        enable_asserts=False,
        num_devices=num_cores,
    )

    x_tbl = nc.dram_tensor("x_tbl", [n_nodes, XCOLS], F32, kind="ExternalInput")
    src_g = nc.dram_tensor("src_g", [nsuper * 128, 4], I32, kind="ExternalInput")
    shpat = nc.dram_tensor("shpat", [nsuper * 128, 4 * XCOLS], BF16,
                           kind="ExternalInput")
    shvrep = nc.dram_tensor("shvrep", [nsuper * 128, 4 * 48], BF16,
                            kind="ExternalInput")
    smat = nc.dram_tensor("smat", [nsuper * 128, 4 * 128], BF16,
                          kind="ExternalInput")
    rbf17 = nc.dram_tensor("rbf17", [RBF + 1, epad], BF16, kind="ExternalInput")
    w1b = nc.dram_tensor("w1b", [RBF + 1, HID], BF16, kind="ExternalInput")
    w2b = nc.dram_tensor("w2b", [HID + 1, HID], BF16, kind="ExternalInput")
    w3p = nc.dram_tensor("w3p", [HID + 1, WNUMEL], BF16, kind="ExternalInput")
    g12 = nc.dram_tensor("g12", [npc_pad, 2], I32, kind="ExternalInput")
    w12 = nc.dram_tensor("w12", [npc_pad, 2], F32, kind="ExternalInput")

    yN = nc.dram_tensor("yN", [npc_pad, FDIM], F32, kind="ExternalOutput")
    partials = nc.dram_tensor("partials", [epad, FDIM], F32)

    with tile.TileContext(nc) as tc:
        with (
            nc.allow_low_precision(reason="bf16 per-edge messages, f32 agg"),
            tc.tile_pool(name="const", bufs=1) as cp,
        ):
            w1b_sb = cp.tile([RBF + 1, HID], BF16)
            nc.sync.dma_start(out=w1b_sb[:], in_=w1b[:])
            w2b_sb = cp.tile([HID + 1, HID], BF16)
            nc.sync.dma_start(out=w2b_sb[:], in_=w2b[:])
            w3p_sb = cp.tile([HID + 1, WNUMEL], BF16)
            nc.sync.dma_start(out=w3p_sb[:], in_=w3p[:])

            # ---------------- pass 1: edges ----------------
            with (
                tc.tile_pool(name="sb", bufs=2) as sp,
                tc.tile_pool(name="wps", bufs=4, space="PSUM") as wpp,
                tc.tile_pool(name="mlp", bufs=2, space="PSUM") as mpp,
                tc.tile_pool(name="cmb", bufs=2, space="PSUM") as cpp,
            ):
                for s in range(nsuper):
                    r0 = s * 512
                    rbf_t = sp.tile([RBF + 1, 512], BF16, tag="rbf")
                    nc.sync.dma_start(out=rbf_t[:], in_=rbf17[:, r0:r0 + 512])
                    src_t = sp.tile([128, 4], I32, tag="src")
                    nc.sync.dma_start(out=src_t[:],
                                      in_=src_g[s * 128:(s + 1) * 128, :])
                    shp_t = sp.tile([128, 4 * XCOLS], BF16, tag="shp")
                    nc.sync.dma_start(out=shp_t[:],
                                      in_=shpat[s * 128:(s + 1) * 128, :])
                    shv_t = sp.tile([128, 4 * 48], BF16, tag="shv")
                    nc.sync.dma_start(out=shv_t[:],
                                      in_=shvrep[s * 128:(s + 1) * 128, :])
                    s_t = sp.tile([128, 4 * 128], BF16, tag="smat")
                    nc.sync.dma_start(out=s_t[:],
                                      in_=smat[s * 128:(s + 1) * 128, :])

                    # radial MLP
                    h1_ps = mpp.tile([HID, 512], F32, tag="mlp")
                    nc.tensor.matmul(out=h1_ps[:], lhsT=w1b_sb[:], rhs=rbf_t[:],
                                     start=True, stop=True)
                    h1s = sp.tile([HID + 1, 512], BF16, tag="h1s")
                    nc.scalar.activation(h1s[:HID, :], h1_ps[:], AF.Silu)
                    nc.gpsimd.memset(h1s[HID:HID + 1, :], 1.0)
                    h2_ps = mpp.tile([HID, 512], F32, tag="mlp")
                    nc.tensor.matmul(out=h2_ps[:], lhsT=w2b_sb[:], rhs=h1s[:],
                                     start=True, stop=True)
                    h2s = sp.tile([HID + 1, 512], BF16, tag="h2s")
                    nc.scalar.activation(h2s[:HID, :], h2_ps[:], AF.Silu)
                    nc.gpsimd.memset(h2s[HID:HID + 1, :], 1.0)

                    # gather + W' per sub-tile
                    xg = sp.tile([128, 4 * XCOLS], F32, tag="xg")
                    wsb = sp.tile([128, 4 * WNUMEL], BF16, tag="wsb")
                    for c in range(4):
                        nc.gpsimd.indirect_dma_start(
                            out=xg[:, c * XCOLS:(c + 1) * XCOLS],
                            out_offset=None, in_=x_tbl[:],
                            in_offset=bass.IndirectOffsetOnAxis(
                                ap=src_t[:, c:c + 1], axis=0))
                        lhs = h2s[:, c * 128:(c + 1) * 128]
                        for k in range(5):
                            q0 = k * 512
                            q1 = min(q0 + 512, WNUMEL)
                            w_ps = wpp.tile([128, 512], F32, tag="w")
                            nc.tensor.matmul(out=w_ps[:, :q1 - q0], lhsT=lhs,
                                             rhs=w3p_sb[:, q0:q1],
                                             start=True, stop=True)
                            nc.scalar.copy(
                                wsb[:, c * WNUMEL + q0:c * WNUMEL + q1],
                                w_ps[:, :q1 - q0])

                    # ---- supertile-batched DVE ----
                    # xft = xg * shpat  (f32 x bf16 -> bf16)
                    xft = sp.tile([128, 4 * XCOLS], BF16, tag="xft")
                    nc.vector.tensor_tensor(out=xft[:], in0=xg[:], in1=shp_t[:],
                                            op=OP.mult)
                    xft4 = xft[:].rearrange("p (s f) -> p s f", f=XCOLS)
                    # af_b[i] = sum_d xv(i,d)*sh_v(d) -> xft cols 32:48
                    nc.vector.tensor_reduce(
                        out=xft4[:, :, 32:48],
                        in_=xft4[:, :, 96:144].rearrange(
                            "p s (i d) -> p s i d", d=3),
                        axis=mybir.AxisListType.X, op=OP.add)

                    wsb4 = wsb[:].rearrange("p (s q) -> p s q", q=WNUMEL)
                    # products
                    pa = sp.tile([128, 4 * 1536], BF16, tag="pa")
                    nc.vector.tensor_tensor(
                        out=pa[:].rearrange("p (s o j) -> p s o j", o=32, j=48),
                        in0=wsb4[:, :, 0:1536].rearrange(
                            "p s (o j) -> p s o j", j=48),
                        in1=xft4[:, :, 0:48].unsqueeze(2)
                            .to_broadcast([128, 4, 32, 48]),
                        op=OP.mult)
                    pb = sp.tile([128, 4 * 512], BF16, tag="pb")
                    nc.vector.tensor_tensor(
                        out=pb[:].rearrange("p (s o i) -> p s o i", o=16, i=32),
                        in0=wsb4[:, :, 1536:2048].rearrange(
                            "p s (o i) -> p s o i", i=32),
                        in1=xft4[:, :, 144:176].unsqueeze(2)
                            .to_broadcast([128, 4, 16, 32]),
                        op=OP.mult)
                    pc = sp.tile([128, 4 * 768], BF16, tag="pc")
                    for c in range(4):
                        nc.vector.tensor_tensor(
                            out=pc[:, c * 768:(c + 1) * 768].rearrange(
                                "p (d o i) -> p d o i", d=3, i=16),
                            in0=wsb[:, c * WNUMEL + 2048:c * WNUMEL + 2304]
                                .rearrange("p (o i) -> p o i", i=16)
                                .unsqueeze(1).to_broadcast([128, 3, 16, 16]),
                            in1=xft[:, c * XCOLS + 48:c * XCOLS + 96]
                                .rearrange("p (d i) -> p d i", i=16)
                                .unsqueeze(2).to_broadcast([128, 3, 16, 16]),
                            op=OP.mult)

                    m_t = sp.tile([128, 4 * FDIM], BF16, tag="m")
                    m4 = m_t[:].rearrange("p (s f) -> p s f", f=FDIM)

                    # A tree: 48 -> 24 -> 12 -> 6 -> 3 -> reduce3
                    pa4 = pa[:].rearrange("p (s o j) -> p s o j", o=32, j=48)
                    ta1 = sp.tile([128, 4 * 768], BF16, tag="ta1")
                    t1v = ta1[:].rearrange("p (s o j) -> p s o j", o=32, j=24)
                    nc.vector.tensor_tensor(out=t1v, in0=pa4[:, :, :, 0:24],
                                            in1=pa4[:, :, :, 24:48], op=OP.add)
                    ta2 = sp.tile([128, 4 * 384], BF16, tag="ta2")
                    t2v = ta2[:].rearrange("p (s o j) -> p s o j", o=32, j=12)
                    nc.vector.tensor_tensor(out=t2v, in0=t1v[:, :, :, 0:12],
                                            in1=t1v[:, :, :, 12:24], op=OP.add)
                    ta3 = sp.tile([128, 4 * 192], BF16, tag="ta3")
                    t3v = ta3[:].rearrange("p (s o j) -> p s o j", o=32, j=6)
                    nc.vector.tensor_tensor(out=t3v, in0=t2v[:, :, :, 0:6],
                                            in1=t2v[:, :, :, 6:12], op=OP.add)
                    ta4 = sp.tile([128, 4 * 96], BF16, tag="ta4")
                    t4v = ta4[:].rearrange("p (s o j) -> p s o j", o=32, j=3)
                    nc.vector.tensor_tensor(out=t4v, in0=t3v[:, :, :, 0:3],
                                            in1=t3v[:, :, :, 3:6], op=OP.add)
                    nc.vector.tensor_reduce(
                        out=m4[:, :, 0:32], in_=t4v,
                        axis=mybir.AxisListType.X, op=OP.add)

                    # B tree: 32 -> 16 -> 8 -> 4 -> 2 -> add
                    pb4 = pb[:].rearrange("p (s o i) -> p s o i", o=16, i=32)
                    tb1 = sp.tile([128, 4 * 256], BF16, tag="tb1")
                    b1v = tb1[:].rearrange("p (s o i) -> p s o i", o=16, i=16)
                    nc.vector.tensor_tensor(out=b1v, in0=pb4[:, :, :, 0:16],
                                            in1=pb4[:, :, :, 16:32], op=OP.add)
                    tb2 = sp.tile([128, 4 * 128], BF16, tag="tb2")
                    b2v = tb2[:].rearrange("p (s o i) -> p s o i", o=16, i=8)
                    nc.vector.tensor_tensor(out=b2v, in0=b1v[:, :, :, 0:8],
                                            in1=b1v[:, :, :, 8:16], op=OP.add)
                    tb3 = sp.tile([128, 4 * 64], BF16, tag="tb3")
                    b3v = tb3[:].rearrange("p (s o i) -> p s o i", o=16, i=4)
                    nc.vector.tensor_tensor(out=b3v, in0=b2v[:, :, :, 0:4],
                                            in1=b2v[:, :, :, 4:8], op=OP.add)
                    tb4 = sp.tile([128, 4 * 32], BF16, tag="tb4")
                    b4v = tb4[:].rearrange("p (s o i) -> p s o i", o=16, i=2)
                    nc.vector.tensor_tensor(out=b4v, in0=b3v[:, :, :, 0:2],
                                            in1=b3v[:, :, :, 2:4], op=OP.add)
                    t2s = sp.tile([128, 4 * 16], BF16, tag="t2s")
                    t2s4 = t2s[:].rearrange("p (s o) -> p s o", o=16)
                    nc.vector.tensor_tensor(out=t2s4, in0=b4v[:, :, :, 0],
                                            in1=b4v[:, :, :, 1], op=OP.add)

                    # C tree: 16 -> 8 -> 4 -> 2 -> add  (groups (s,d,o))
                    pc4 = pc[:].rearrange("p (s g i) -> p s g i", g=48, i=16)
                    tc1 = sp.tile([128, 4 * 384], BF16, tag="tc1")
                    c1v = tc1[:].rearrange("p (s g i) -> p s g i", g=48, i=8)
                    nc.vector.tensor_tensor(out=c1v, in0=pc4[:, :, :, 0:8],
                                            in1=pc4[:, :, :, 8:16], op=OP.add)
                    tc2 = sp.tile([128, 4 * 192], BF16, tag="tc2")
                    c2v = tc2[:].rearrange("p (s g i) -> p s g i", g=48, i=4)
                    nc.vector.tensor_tensor(out=c2v, in0=c1v[:, :, :, 0:4],
                                            in1=c1v[:, :, :, 4:8], op=OP.add)
                    tc3 = sp.tile([128, 4 * 96], BF16, tag="tc3")
                    c3v = tc3[:].rearrange("p (s g i) -> p s g i", g=48, i=2)
                    nc.vector.tensor_tensor(out=c3v, in0=c2v[:, :, :, 0:2],
                                            in1=c2v[:, :, :, 2:4], op=OP.add)
                    v3 = sp.tile([128, 4 * 48], BF16, tag="v3")
                    v34 = v3[:].rearrange("p (s g) -> p s g", g=48)
                    nc.vector.tensor_tensor(out=v34, in0=c3v[:, :, :, 0],
                                            in1=c3v[:, :, :, 1], op=OP.add)

                    # m_v = t2 (x) sh_v  +  v3   (v3 already has sh_s folded)
                    mtmp = sp.tile([128, 4 * 48], BF16, tag="mtmp")
                    nc.vector.tensor_tensor(
                        out=mtmp[:].rearrange("p (s d o) -> p s d o", d=3, o=16),
                        in0=t2s4.unsqueeze(2).to_broadcast([128, 4, 3, 16]),
                        in1=shv_t[:].rearrange("p (s d o) -> p s d o", d=3, o=16),
                        op=OP.mult)
                    nc.vector.tensor_tensor(
                        out=m4[:, :, 32:80],
                        in0=mtmp[:].rearrange("p (s g) -> p s g", g=48),
                        in1=v34, op=OP.add)

                    # S-combine on PE, per sub-tile, into one PSUM bank
                    comb_ps = cpp.tile([128, 4 * FDIM], F32, tag="comb")
                    for c in range(4):
                        nc.tensor.matmul(
                            out=comb_ps[:, c * FDIM:(c + 1) * FDIM],
                            lhsT=s_t[:, c * 128:(c + 1) * 128],
                            rhs=m_t[:, c * FDIM:(c + 1) * FDIM],
                            start=True, stop=True)
                    comb_sb = sp.tile([128, 4 * FDIM], F32, tag="combsb")
                    nc.scalar.copy(comb_sb[:], comb_ps[:])
                    nc.sync.dma_start(
                        out=partials[r0:r0 + 512, :].rearrange(
                            "(c p) f -> p c f", c=4),
                        in_=comb_sb[:].rearrange("p (c f) -> p c f", c=4))

            # ---------------- pass 2: nodes ----------------
            with (
                tc.tile_pool(name="sb2", bufs=4) as s2,
            ):
                for b in range(nb):
                    n0 = b * 128
                    g_t = s2.tile([128, 2], I32, tag="g")
                    nc.sync.dma_start(out=g_t[:], in_=g12[n0:n0 + 128, :])
                    wv_t = s2.tile([128, 2], F32, tag="wv")
                    nc.sync.dma_start(out=wv_t[:], in_=w12[n0:n0 + 128, :])
                    p1 = s2.tile([128, FDIM], F32, tag="p1")
                    nc.gpsimd.indirect_dma_start(
                        out=p1[:], out_offset=None, in_=partials[:],
                        in_offset=bass.IndirectOffsetOnAxis(
                            ap=g_t[:, 0:1], axis=0))
                    p2 = s2.tile([128, FDIM], F32, tag="p2")
                    nc.gpsimd.indirect_dma_start(
                        out=p2[:], out_offset=None, in_=partials[:],
                        in_offset=bass.IndirectOffsetOnAxis(
                            ap=g_t[:, 1:2], axis=0))
                    agg = s2.tile([128, FDIM], F32, tag="agg")
                    nc.vector.tensor_scalar(
                        out=agg[:], in0=p1[:], scalar1=wv_t[:, 0:1],
                        scalar2=None, op0=OP.mult)
                    agg2 = s2.tile([128, FDIM], F32, tag="agg2")
                    nc.vector.tensor_scalar(
                        out=agg2[:], in0=p2[:], scalar1=wv_t[:, 1:2],
                        scalar2=None, op0=OP.mult)
                    nc.vector.tensor_tensor(out=agg[:], in0=agg[:],
                                            in1=agg2[:], op=OP.add)
                    nc.sync.dma_start(out=yN[n0:n0 + 128, :], in_=agg[:])

    nc.compile()
    return nc


_PROGRAM_CACHE = {}


def _get_program(n_nodes, npc_pad, epad, num_cores):
    key = (n_nodes, npc_pad, epad, num_cores)
    if key not in _PROGRAM_CACHE:
        _PROGRAM_CACHE[key] = build_program(n_nodes, npc_pad, epad, num_cores)
    return _PROGRAM_CACHE[key]


def prepare_in_maps(x, edge_src, edge_dst, edge_sh, edge_rbf,
                    w1, b1, w2, b2, w3, b3, ws_out, wv_out, num_cores=NCORES):
    n = x.shape[0]
    npc = -(-n // num_cores)
    npc_pad = -(-npc // 128) * 128

    dst = np.asarray(edge_dst, np.int64)
    src = np.asarray(edge_src, np.int64)
    order = np.argsort(dst, kind="stable")
    dst_s = dst[order]
    src_s = src[order]
    sh_s = np.asarray(edge_sh, np.float32)[order]
    rbf_s = np.asarray(edge_rbf, np.float32)[order]

    bounds = np.searchsorted(dst_s, np.arange(num_cores + 1) * npc)
    counts = np.diff(bounds)
    epad = max(512, int(-(-counts.max() // 512) * 512))
    nsuper = epad // 512

    bf16 = mybir.dt.np(BF16)
    w1bh = np.concatenate([np.asarray(w1, np.float32),
                           np.asarray(b1, np.float32)[None, :]], 0).astype(bf16)
    w2bh = np.concatenate([np.asarray(w2, np.float32),
                           np.asarray(b2, np.float32)[None, :]], 0).astype(bf16)
    perm, scl = _build_w3_perm()
    w3p_f = np.concatenate(
        [np.asarray(w3, np.float32)[:, perm] * scl[None, :],
         (np.asarray(b3, np.float32)[perm] * scl)[None, :]], 0)
    # fold the output irrep-linear (Mout) into the per-edge TP weights:
    # block A (s-out): mix o with ws_out; blocks B/C (v-out): mix with wv_out
    wso = np.asarray(ws_out, np.float32)
    wvo = np.asarray(wv_out, np.float32)
    A = w3p_f[:, 0:1536].reshape(HID + 1, 32, 48)
    w3p_f[:, 0:1536] = np.einsum("hoj,oq->hqj", A, wso).reshape(HID + 1, 1536)
    B = w3p_f[:, 1536:2048].reshape(HID + 1, 16, 32)
    w3p_f[:, 1536:2048] = np.einsum("hoi,oq->hqi", B, wvo).reshape(HID + 1, 512)
    C = w3p_f[:, 2048:2304].reshape(HID + 1, 16, 16)
    w3p_f[:, 2048:2304] = np.einsum("hoi,oq->hqi", C, wvo).reshape(HID + 1, 256)
    w3ph = w3p_f.astype(bf16)
    xf = np.asarray(x, np.float32)

    # x table: [xs 32 | zeros 16 | xv(d,i) 48 | xv(i,d) 48 | xs 32]
    xtbl = np.zeros((n, XCOLS), np.float32)
    xtbl[:, :MUL0] = xf[:, :MUL0]
    xv = xf[:, MUL0:].reshape(n, MUL1, 3)         # (i, d)
    xtbl[:, 48:96] = xv.transpose(0, 2, 1).reshape(n, 48)   # (d, i)
    xtbl[:, 96:144] = xf[:, MUL0:]                           # (i, d)
    xtbl[:, 144:176] = xf[:, :MUL0]

    in_maps = []
    meta = {"npc": npc, "npc_pad": npc_pad, "epad": epad, "n": n,
            "num_cores": num_cores}
    for c in range(num_cores):
        lo, hi = bounds[c], bounds[c + 1]
        ec = hi - lo
        csrc = np.zeros(epad, np.int32)
        csrc[:ec] = src_s[lo:hi]
        cdst = np.full(epad, -1, np.int64)
        cdst[:ec] = dst_s[lo:hi]
        csh = np.zeros((epad, 4), np.float32)
        csh[:ec] = sh_s[lo:hi]

        crbf = np.zeros((RBF + 1, epad), np.float32)
        crbf[:RBF, :ec] = rbf_s[lo:hi].T
        crbf[RBF, :] = 1.0
        crbf = crbf.astype(bf16)

        # per-edge scale pattern [epad, 144]:
        # [sh_s*32 | 0*16 | sh_s*48 | sh_v(i,d)*48]
        pat = np.zeros((epad, XCOLS), np.float32)
        pat[:, 0:32] = csh[:, 0:1]
        pat[:, 48:96] = csh[:, 0:1]
        pat[:, 96:144] = np.tile(csh[:, 1:4], (1, 16))
        pat[:ec, 144:176] = 1.0
        # shv repeated in (d, o) layout
        svr = np.repeat(csh[:, 1:4], 16, axis=1)  # [epad, 48]

        # S matrices per 128-edge tile
        d2 = cdst.reshape(-1, 128)
        S = (d2[:, :, None] == d2[:, None, :]) & (d2[:, :, None] >= 0)
        S = S.astype(np.float32)

        # reorder edge-major [epad] -> [nsuper, 128, 4(sub)]
        def to_g(a, width):
            a = a.reshape(nsuper, 4, 128, width)
            return a.transpose(0, 2, 1, 3).reshape(nsuper * 128, 4 * width)

        csrc_g = to_g(csrc.reshape(epad, 1), 1).astype(np.int32)
        pat_g = to_g(pat, XCOLS).astype(bf16)
        svr_g = to_g(svr, 48).astype(bf16)
        S_g = to_g(S.reshape(epad, 128), 128).astype(bf16)

        # node -> first/last edge rows (local), inv-degree folded weights
        nbase = c * npc
        nodes = np.arange(npc_pad, dtype=np.int64) + nbase
        first = np.searchsorted(dst_s[lo:hi], nodes, side="left")
        last = np.searchsorted(dst_s[lo:hi], nodes, side="right") - 1
        deg = (last - first + 1).astype(np.int64)
        has = deg > 0
        g = np.zeros((npc_pad, 2), np.int32)
        wv = np.zeros((npc_pad, 2), np.float32)
        g[has, 0] = first[has].astype(np.int32)
        g[has, 1] = last[has].astype(np.int32)
        inv = np.zeros(npc_pad, np.float32)
        inv[has] = 1.0 / deg[has]
        wv[has, 0] = inv[has]
        wv[has, 1] = ((first[has] // 128) != (last[has] // 128)) * inv[has]

        in_maps.append({
            "x_tbl": xtbl, "src_g": csrc_g, "shpat": pat_g,
            "shvrep": svr_g, "smat": S_g, "rbf17": crbf,
            "w1b": w1bh, "w2b": w2bh, "w3p": w3ph,
            "g12": g, "w12": wv,
        })
    return in_maps, meta


def kernel(x, edge_src, edge_dst, edge_sh, edge_rbf,
           w1, b1, w2, b2, w3, b3, ws_self, wv_self, ws_out, wv_out,
           _trace=False):
    num_cores = NCORES
    in_maps, meta = prepare_in_maps(
        x, edge_src, edge_dst, edge_sh, edge_rbf, w1, b1, w2, b2, w3, b3,
        ws_out, wv_out, num_cores=num_cores)

    nc = _get_program(meta["n"], meta["npc_pad"], meta["epad"], num_cores)
    res = run_bass_kernel_spmd(nc, in_maps, list(range(num_cores)),
                               trace=_trace)

    # self path in f32 on host (exact)
    xf = np.asarray(x, np.float32)
    n, npc = meta["n"], meta["npc"]
    ys = xf[:, :MUL0] @ np.asarray(ws_self, np.float32)
    yv = np.einsum("nid,io->nod", xf[:, MUL0:].reshape(n, MUL1, 3),
                   np.asarray(wv_self, np.float32))
    y = np.concatenate([ys, yv.reshape(n, 3 * MUL1)], axis=1)
    for c in range(num_cores):
        lo = c * npc
        hi = min(lo + npc, n)
        aggc = np.asarray(res.results[c]["yN"])[:hi - lo]
        # agg layout L: [s(32) | v(d,o)] -> reference cols [s | v(o,d)]
        y[lo:hi, :MUL0] += aggc[:, :MUL0]
        v = aggc[:, MUL0:].reshape(hi - lo, 3, MUL1)
        y[lo:hi, MUL0:] += v.transpose(0, 2, 1).reshape(hi - lo, 48)
    kernel._last_results = res
    return y
